# revision 47
# baseline (speedup 1.0000x reference)
"""Trainium2 Bass kernel for nn_EncoderDecoderTransformer (sparse kNN encoder attention).

Sharding: data-parallel over batch (4 batches x 2 cores) with each pair of cores
splitting the sequence dimension (512 tokens each). Per layer, K/V are exchanged
within the pair via AllGather over replica groups [[0,1],[2,3],[4,5],[6,7]].

Layouts (per core):
  - Activations feature-major: x^T stored as 4 tiles (128 dims, 512 own tokens).
  - Q^T/K^T feature-major (head h lives in rows [64*(h%2):...] of ptile h//2).
  - V token-major (128 tokens, 8 heads, 65) with a constant-1 column per head so
    the AV matmul also produces the softmax denominator in psum row 64.
  - Scores computed transposed: S^T = K^T.T @ Q^T  (keys on partitions), exp on
    the Scalar engine, kNN/causal masking as a 0/1 bf16 multiply.
  - kNN mask: s'_qk = 2 x_q.x_k - |x_k|^2 orders like -distance. The 17th
    largest per row (self is always rank 1) is the inclusion threshold; computed
    with the DVE max8/match_replace top-k primitives. Verified to reproduce the
    reference _knn_mask exactly.
"""

import os
import numpy as np
import ml_dtypes

BF16 = ml_dtypes.bfloat16

D, F, H, NE, ND, KNN = 512, 2048, 8, 4, 4, 16
B, LE, LD = 4, 1024, 1024
DH = D // H
NCORE = 8
P = 128
TOWN = 512          # tokens owned per core
NDT = D // P        # 4 feature tiles
NKT = LE // P       # 8 key tiles
NEG = -1e30
EPS = 1e-5
PAIRS = [[0, 1], [2, 3], [4, 5], [6, 7]]

_CACHE = {}


def build(n_enc=NE, n_dec=ND):
    from contextlib import ExitStack

    import concourse.bacc as bacc
    import concourse.tile as tile
    import concourse.mybir as mybir

    f32 = mybir.dt.float32
    bf16 = mybir.dt.bfloat16
    f16 = mybir.dt.float16
    AF = mybir.ActivationFunctionType
    OP = mybir.AluOpType

    import concourse.bass as cbass

    nc = bacc.Bacc("TRN2", target_bir_lowering=False, debug=False, num_devices=NCORE)
    NOGP = not os.environ.get("KQ_GP")  # gpsimd elementwise corrupts data on HW; keep off

    # ---- I/O ----
    def din(name, shape, dt=f32):
        return nc.dram_tensor(name, shape, dt, kind="ExternalInput")

    x0T = din("x0T", [NDT, P, TOWN])
    y0T = din("y0T", [NDT, P, TOWN])
    xq2_d = din("xq2", [TOWN, 3])       # 2*xyz for own tokens
    xq2row_d = din("xq2row", [3, TOWN])  # same, transposed
    xkn_d = din("xkn", [LE, 4])          # [xyz, |xyz|^2] all tokens
    xrow_d = din("xrow", [4, LE])        # same, transposed
    bosrow = din("bosrow", [1, TOWN])
    causal_in = din("causal", [NKT, P, TOWN], bf16)

    ew_qkv = din("ew_qkv", [NE, D, 3 * D], bf16)
    ew_out = din("ew_out", [NE, D, D], bf16)
    ew_f1 = din("ew_f1", [NE, D, F], bf16)
    ew_f2 = din("ew_f2", [NE, F, D], bf16)
    eb_qkv = din("eb_qkv", [NE, 3 * D, 1])
    eb_out = din("eb_out", [NE, D, 1])
    eb_f1 = din("eb_f1", [NE, F, 1])
    eb_f2 = din("eb_f2", [NE, D, 1])

    dw_saqkv = din("dw_saqkv", [ND, D, 3 * D], bf16)
    db_saqkv = din("db_saqkv", [ND, 3 * D, 1])
    dw_saout = din("dw_saout", [ND, D, D], bf16)
    db_saout = din("db_saout", [ND, D, 1])
    dw_caqkv = din("dw_caqkv", [ND, D, 3 * D], bf16)
    db_caqkv = din("db_caqkv", [ND, 3 * D, 1])
    dw_caout = din("dw_caout", [ND, D, D], bf16)
    db_caout = din("db_caout", [ND, D, 1])
    dw_f1 = din("dw_f1", [ND, D, F], bf16)
    db_f1 = din("db_f1", [ND, F, 1])
    dw_f2 = din("dw_f2", [ND, F, D], bf16)
    db_f2 = din("db_f2", [ND, D, 1])
    eb_qkv_bf = din("eb_qkv_bf", [NE, 3 * D, 1], bf16)
    db_saqkv_bf = din("db_saqkv_bf", [ND, 3 * D, 1], bf16)
    db_caqkv_bf = din("db_caqkv_bf", [ND, 3 * D, 1], bf16)

    enc_part = nc.dram_tensor("enc_part", [NDT, P, TOWN], f32, kind="ExternalOutput")
    dec_part = nc.dram_tensor("dec_part", [NDT, P, TOWN], f32, kind="ExternalOutput")
    n_dbg = int(os.environ.get("KQ_DEBUG", "0"))
    dbg_t = None
    if n_dbg:
        dbg_t = nc.dram_tensor("dbg", [n_dbg, NDT, P, TOWN], f32, kind="ExternalOutput")
    dbg_i = [0]
    dbgkv_t = None
    if os.environ.get("KQ_DEBUG_KV"):
        dbgkv_t = nc.dram_tensor("dbgkv", [2, 2, 4, P, TOWN], bf16, kind="ExternalOutput")
        dbgk_t = nc.dram_tensor("dbgk", [4, P, 2, TOWN], bf16, kind="ExternalOutput")
        dbgko_t = nc.dram_tensor("dbgko", [4, P, TOWN], bf16, kind="ExternalOutput")
    dbga_t = None
    if os.environ.get("KQ_DEBUG_ALLOW"):
        dbga_t = nc.dram_tensor("dbga", [NKT, P, TOWN], bf16, kind="ExternalOutput")
        dbgs_t = nc.dram_tensor("dbgs", [NKT, P, TOWN], f32, kind="ExternalOutput")
        dbgt_t = nc.dram_tensor("dbgt", [1, TOWN], f32, kind="ExternalOutput")

    with tile.TileContext(nc) as tc, ExitStack() as ctx:
        ep = ctx.enter_context

        pc = ep(tc.tile_pool(name="pc", bufs=1))
        p_allow = ep(tc.tile_pool(name="p_allow", bufs=8))
        p_causal = ep(tc.tile_pool(name="p_causal", bufs=8))
        # PSUM budget (8 banks): ps_s pair-tiles [P,1024] x2 = 4, ps_o [65,512] x2 = 2,
        # ps_mm [*,512] x2 = 2.
        ps_s = ep(tc.tile_pool(name="ps_s", bufs=2, space="PSUM"))
        ps_o = ep(tc.tile_pool(name="ps_o", bufs=2, space="PSUM"))
        ps_mm = ep(tc.tile_pool(name="ps_mm", bufs=2, space="PSUM"))
        p_dram = ep(tc.tile_pool(name="p_dram", bufs=2, space="DRAM"))

        # ---- constants ----
        ones_col = pc.tile([P, 1], f32)          # LN partition sums (lhsT)
        nc.vector.memset(ones_col, 1.0)
        ones_row = pc.tile([1, P], f32)          # broadcast lhsT (K=1)
        nc.vector.memset(ones_row, 1.0)
        ones_row512 = pc.tile([1, TOWN], f32)    # rhs for row-rank1 bias matmuls
        nc.vector.memset(ones_row512, 1.0)
        ones_col_bf = pc.tile([P, 1], bf16)
        nc.vector.memset(ones_col_bf, 1.0)
        ones_row_bf = pc.tile([1, P], bf16)
        nc.vector.memset(ones_row_bf, 1.0)
        ones_row512_bf = pc.tile([1, TOWN], bf16)
        nc.vector.memset(ones_row512_bf, 1.0)
        ones_row_f16 = pc.tile([1, P], f16)
        nc.vector.memset(ones_row_f16, 1.0)
        dummy_w = pc.tile([P, P], bf16)
        nc.vector.memset(dummy_w, 0.0)
        dummy_x = pc.tile([P, TOWN], bf16)
        nc.vector.memset(dummy_x, 0.0)
        eps_sb = pc.tile([1, 1], f32)
        nc.vector.memset(eps_sb, EPS)

        bos_sb = pc.tile([1, TOWN], f32)
        nc.sync.dma_start(out=bos_sb, in_=bosrow[:, :])

        causal_sb = []
        for kt in range(NKT):
            t = p_causal.tile([P, TOWN], bf16, tag="causal")
            nc.sync.dma_start(out=t, in_=causal_in[kt])
            causal_sb.append(t)

        def dbg(ts):
            if dbg_t is None or dbg_i[0] >= n_dbg:
                return
            for dt in range(NDT):
                nc.sync.dma_start(out=dbg_t[dbg_i[0], dt], in_=ts[dt])
            dbg_i[0] += 1

        def pe_warm(n):
            # filler matmuls emitted into PE-idle windows (AG waits, mask
            # phase) so the HAM clock gate stays at full rate; never read
            psD = ps_mm.tile([P, TOWN], f32, tag="mm")
            for _ in range(n):
                nc.tensor.matmul(psD, dummy_w, dummy_x, start=True, stop=True)

        def build_mask():

            # s'_qk = 2 x_q . x_k - |x_k|^2 computed with IEEE-exact fp32 DVE ops
            # (the PE fp32 matmul is not exact fp32 and flips kNN boundary choices).
            # Both layouts use the same per-element op chain => bit-identical values.
            def bcast_rows(dram_row_ap, pool, n_free, tag):
                # (n_free,) DRAM row -> (P, n_free) SBUF tile, replicated across partitions
                t = pool.tile([P, n_free], f32, tag=tag)
                src_ap = cbass.AP(
                    tensor=dram_row_ap.tensor, offset=dram_row_ap.offset,
                    ap=[[0, P]] + list(dram_row_ap.ap),
                )
                nc.sync.dma_start(out=t, in_=src_ap)
                return t

            allow_sb = []
            with tc.tile_pool(name="p_mask", bufs=2) as p_mask, \
                 tc.tile_pool(name="p_mbc", bufs=1) as p_mbc, \
                 tc.tile_pool(name="p_m8", bufs=8) as p_m8:
                bcx = []
                for c in range(4):
                    t = bcast_rows(xrow_d[c], p_mbc, LE, tag=f"bcx{c}")
                    bcx.append(t)
                tcol_dram = p_dram.tile([4, P, 1], f32, tag="tcol")
                for qt in range(4):
                    xqc = p_m8.tile([P, 3], f32, tag="xqc")
                    nc.sync.dma_start(out=xqc, in_=xq2_d[qt * P:(qt + 1) * P, :])
                    s0 = p_mask.tile([P, LE], f32, tag="s")
                    nc.vector.tensor_scalar(s0, bcx[0], xqc[:, 0:1], None, op0=OP.mult)
                    s1 = p_mask.tile([P, LE], f32, tag="s")
                    nc.vector.scalar_tensor_tensor(s1, bcx[1], xqc[:, 1:2], s0, OP.mult, OP.add)
                    s2 = p_mask.tile([P, LE], f32, tag="s")
                    nc.vector.scalar_tensor_tensor(s2, bcx[2], xqc[:, 2:3], s1, OP.mult, OP.add)
                    s3 = p_mask.tile([P, LE], f32, tag="s")
                    nc.vector.tensor_tensor(s3, s2, bcx[3], OP.subtract)
                    m8 = p_m8.tile([P, 8], f32, tag="m8")
                    nc.vector.max(m8, s3)
                    s4 = p_mask.tile([P, LE], f32, tag="s")
                    nc.vector.match_replace(s4, m8, s3, NEG)
                    m8b = p_m8.tile([P, 8], f32, tag="m8")
                    nc.vector.max(m8b, s4)
                    s5 = p_mask.tile([P, LE], f32, tag="s")
                    nc.vector.match_replace(s5, m8b, s4, NEG)
                    m8c = p_m8.tile([P, 8], f32, tag="m8")
                    nc.vector.max(m8c, s5)
                    # rank-17 value (16 NN + self) is the inclusion threshold
                    nc.sync.dma_start(out=tcol_dram[qt], in_=m8c[:, 0:1])
                t_row = pc.tile([1, TOWN], f32)
                nc.sync.dma_start(
                    out=t_row, in_=tcol_dram.rearrange("a p one -> one (a p)")
                )
                t2 = pc.tile([1, TOWN], f32)
                nc.vector.tensor_tensor(t2, t_row, bos_sb, OP.min)
                t2_dram = p_dram.tile([1, TOWN], f32, tag="t2d")
                nc.sync.dma_start(out=t2_dram, in_=t2)
                t_bc = bcast_rows(t2_dram[0], pc, TOWN, tag="t_bc")
                bq = []
                for c in range(3):
                    t = bcast_rows(xq2row_d[c], p_mbc, TOWN, tag=f"bq{c}")
                    bq.append(t)
                for kt in range(NKT):
                    xkc = p_m8.tile([P, 4], f32, tag="xkc")
                    nc.sync.dma_start(out=xkc, in_=xkn_d[kt * P:(kt + 1) * P, :])
                    u0 = p_mask.tile([P, TOWN], f32, tag="st")
                    nc.vector.tensor_scalar(u0, bq[0], xkc[:, 0:1], None, op0=OP.mult)
                    u1 = p_mask.tile([P, TOWN], f32, tag="st")
                    nc.vector.scalar_tensor_tensor(u1, bq[1], xkc[:, 1:2], u0, OP.mult, OP.add)
                    u2 = p_mask.tile([P, TOWN], f32, tag="st")
                    nc.vector.scalar_tensor_tensor(u2, bq[2], xkc[:, 2:3], u1, OP.mult, OP.add)
                    u3 = p_mask.tile([P, TOWN], f32, tag="st")
                    nc.vector.tensor_scalar(u3, u2, xkc[:, 3:4], None, op0=OP.subtract)
                    al = p_allow.tile([P, TOWN], bf16, tag="allow")
                    nc.vector.tensor_tensor(al, u3, t_bc, OP.is_ge)
                    if kt == 0:
                        # BOS key allowed for all q; emitted here (not after the
                        # loop) so attention on kt=0 can start immediately
                        nc.vector.memset(al[0:1, :], 1.0)
                    if dbga_t is not None:
                        nc.sync.dma_start(out=dbgs_t[kt], in_=u3)
                    allow_sb.append(al)
                if dbga_t is not None:
                    nc.sync.dma_start(out=dbgt_t[:, :], in_=t2)
                    for kt in range(NKT):
                        nc.sync.dma_start(out=dbga_t[kt], in_=allow_sb[kt])
            return allow_sb



        # ================= helpers =================
        def load_w(pool, dram_ap, kchunks, cols, tag):
            t = pool.tile([P, kchunks, cols], bf16, tag=tag)
            nc.sync.dma_start(
                out=t, in_=dram_ap.rearrange("(kc p) m -> p kc m", p=P)
            )
            return t

        def xshadow(x):
            # bf16 shadow + square, emitted at x-production time so the next
            # layer_norm starts directly with its reduction matmuls
            xb = p_lnsq.tile([P, TOWN], bf16, tag="lnxb")
            nc.vector.tensor_copy(xb, x)
            sq = p_lnsq.tile([P, TOWN], bf16, tag="lnsq")
            nc.scalar.activation(sq, x, AF.Square)
            return xb, sq

        def layer_norm(xst, out_dt, out_pool, out_tag):
            xs, xbs, sqs = xst
            ps_mean = ps_mm.tile([1, TOWN], f32, tag="mm")
            for dt in range(NDT):
                nc.tensor.matmul(ps_mean, ones_col_bf, xbs[dt], start=dt == 0, stop=dt == 3)
            ps_sq = ps_mm.tile([1, TOWN], f32, tag="mm")
            for dt in range(NDT):
                nc.tensor.matmul(ps_sq, ones_col_bf, sqs[dt], start=dt == 0, stop=dt == 3)
            mu16 = p_small.tile([1, TOWN], f16, tag="smf16")
            nc.scalar.mul(mu16, ps_mean, 1.0 / D)
            musq = p_small.tile([1, TOWN], f32, tag="sm")
            nc.scalar.activation(musq, ps_mean, AF.Square, scale=1.0 / D)
            var = p_small.tile([1, TOWN], f32, tag="sm")
            nc.vector.scalar_tensor_tensor(var, ps_sq, 1.0 / D, musq, OP.mult, OP.subtract)
            lnv = p_small.tile([1, TOWN], f32, tag="sm")
            nc.scalar.activation(lnv, var, AF.Ln, bias=eps_sb)
            rstd = p_small.tile([1, TOWN], f16, tag="smf16")
            nc.scalar.activation(rstd, lnv, AF.Exp, scale=-0.5)
            # h = (x - mu)*rstd via f16 rank-1 broadcasts of mu and rstd
            ps_a = ps_mm.tile([P, TOWN], f32, tag="mm")
            nc.tensor.matmul(ps_a, ones_row_f16, rstd, start=True, stop=True)
            ps_c = ps_mm.tile([P, TOWN], f32, tag="mm")
            nc.tensor.matmul(ps_c, ones_row_f16, mu16, start=True, stop=True)
            a_sb = p_lnac.tile([P, TOWN], f32, tag="lna")
            nc.vector.tensor_copy(a_sb, ps_a)
            c_sb = p_lnac.tile([P, TOWN], f32, tag="lnc")
            nc.vector.tensor_copy(c_sb, ps_c)
            hs = []
            for dt in range(NDT):
                h = out_pool.tile([P, TOWN], out_dt, tag=out_tag)
                nc.vector.tensor_tensor(h, xs[dt], c_sb, OP.subtract)
                (nc.vector if NOGP else nc.gpsimd).tensor_tensor(h, h, a_sb, OP.mult)
                hs.append(h)
            return hs

        def proj_fm(w_sb, col_off, n_m, rhs, bias_ap, out_pool, out_tag, out_dt=bf16):
            """Feature-major projection; per-partition bias applied on eviction."""
            outs = []
            nk = len(rhs)
            for m in range(n_m):
                ps = ps_mm.tile([P, TOWN], f32, tag="mm")
                for kc in range(nk):
                    nc.tensor.matmul(
                        ps, w_sb[:, kc, col_off + m * P:col_off + (m + 1) * P],
                        rhs[kc], start=kc == 0, stop=kc == nk - 1,
                    )
                bcol = p_bias.tile([P, 1], f32, tag="bcol")
                nc.sync.dma_start(out=bcol, in_=bias_ap[col_off + m * P:col_off + (m + 1) * P, :])
                o = out_pool.tile([P, TOWN], out_dt, tag=out_tag)
                nc.vector.tensor_scalar(o, ps, bcol, None, op0=OP.add)
                outs.append(o)
            return outs

        def proj_rank1bias(w_sb, col_off, rhs, bias_ap, bias_off, token_major):
            """K^T (feature-major) or V (token-major) projection with the bias
            folded in as a rank-1 bf16 matmul; evicted compact bf16 for the AG bounce."""
            outs = []
            brow512 = None
            if token_major:
                brow512 = p_bias.tile([1, TOWN], bf16, tag="brow512")
                nc.sync.dma_start(
                    out=brow512,
                    in_=bias_ap[bias_off:bias_off + D, :].rearrange("a b -> b a"),
                )
            for m in range(4):
                ps = ps_mm.tile([P, TOWN], f32, tag="mm")
                if token_major:
                    for kc in range(4):
                        nc.tensor.matmul(
                            ps, rhs[kc][:, m * P:(m + 1) * P],
                            w_sb[:, kc, col_off:col_off + D],
                            start=kc == 0, stop=False,
                        )
                    nc.tensor.matmul(ps, ones_row_bf, brow512, start=False, stop=True)
                else:
                    for kc in range(4):
                        nc.tensor.matmul(
                            ps, w_sb[:, kc, col_off + m * P:col_off + (m + 1) * P],
                            rhs[kc], start=kc == 0, stop=False,
                        )
                    brow = p_bias.tile([1, P], bf16, tag="brow")
                    nc.sync.dma_start(
                        out=brow,
                        in_=bias_ap[bias_off + m * P:bias_off + (m + 1) * P, :].rearrange("a b -> b a"),
                    )
                    nc.tensor.matmul(ps, brow, ones_row512_bf, start=False, stop=True)
                o = p_kvout.tile([P, TOWN], bf16, tag="kvout")
                nc.vector.tensor_copy(o, ps)
                outs.append(o)
            return outs

        def kv_allgather(k_outs, v_outs, uniq):
            # one DRAM buffer set per layer: a pool-recycled buffer could be
            # rewritten while the pair-peer's collective still reads it
            # (cross-core hazard invisible to Tile's per-core dep tracking).
            # K and V go in separate AllGathers so attention (which needs K
            # first) can start while V is still in flight.
            bin_k = p_dram.tile([4, P, TOWN], bf16, tag=f"agink{uniq}")
            for p in range(4):
                nc.sync.dma_start(out=bin_k[p], in_=k_outs[p])
            bout_k = p_dram.tile([2, 4, P, TOWN], bf16, tag=f"agoutk{uniq}")
            nc.gpsimd.collective_compute(
                "AllGather", OP.bypass, replica_groups=PAIRS,
                ins=[bin_k[:].opt()], outs=[bout_k[:].opt()],
            )
            bin_v = p_dram.tile([4, P, TOWN], bf16, tag=f"aginv{uniq}")
            for p in range(4):
                nc.sync.dma_start(out=bin_v[p], in_=v_outs[p])
            bout_v = p_dram.tile([2, 4, P, TOWN], bf16, tag=f"agoutv{uniq}")
            nc.gpsimd.collective_compute(
                "AllGather", OP.bypass, replica_groups=PAIRS,
                ins=[bin_v[:].opt()], outs=[bout_v[:].opt()],
            )
            return (bout_k, bout_v)

        def load_k(bout):
            Ks = []
            for p in range(4):
                kt = p_kv.tile([P, 2, TOWN], bf16, tag="ksb")
                for r in range(2):
                    nc.sync.dma_start(out=kt[:, r, :], in_=bout[r, p])
                Ks.append(kt)
            return Ks

        def load_v(bout):
            Vs = []
            for r in range(2):
                for tt in range(4):
                    vt = p_v.tile([P, H, 65], bf16, tag="vsb")
                    nc.sync.dma_start(
                        out=vt[:, :, 0:64],
                        in_=bout[r, tt].rearrange("p (h d) -> p h d", h=H),
                    )
                    (nc.vector if NOGP else nc.gpsimd).memset(vt[:, :, 64:65], 1.0)
                    Vs.append(vt)
            return Vs

        def attention(Qs, Ks, Vs, mask_tiles):
            # Scores for the head pair land in one 2-bank PSUM tile [P, 2*TOWN]
            # (j=0 cols 0:TOWN, j=1 cols TOWN:2*TOWN) so exp and the mask
            # multiply run once per (hp, kt) at N=1024 instead of twice at
            # N=512 — the serial exp->mask chain was the attention rate
            # limiter. psO is evicted to SBUF with a single copy so the two
            # PSUM banks recycle quickly.
            OTs = []
            for hp in range(4):
                psO = []
                for _j in range(2):
                    psO_t = ps_o.tile([65, TOWN], f32, tag="pso")
                    psO.append(psO_t)
                for kt in range(NKT):
                    r, c = kt // 4, kt % 4
                    psP = ps_s.tile([P, 2 * TOWN], f32, tag="pss")
                    for j in range(2):
                        rows = slice(j * 64, (j + 1) * 64)
                        nc.tensor.matmul(
                            psP[:, j * TOWN:(j + 1) * TOWN],
                            Ks[hp][rows, r, c * P:(c + 1) * P], Qs[hp][rows, :],
                            start=True, stop=True,
                        )
                    e = p_e.tile([P, 2 * TOWN], bf16, tag="e")
                    nc.scalar.activation(e, psP, AF.Exp, scale=0.125)
                    if mask_tiles is not None:
                        # one multiply over both halves; the [P, TOWN] mask is
                        # repeated via a stride-0 middle AP dim
                        m_ap = mask_tiles[kt][:, :]
                        m_rep = cbass.AP(
                            tensor=m_ap.tensor, offset=m_ap.offset,
                            ap=[list(m_ap.ap[0]), [0, 2]] + [list(d) for d in m_ap.ap[1:]],
                        )
                        e_ap = e[:, :]
                        e_v = cbass.AP(
                            tensor=e_ap.tensor, offset=e_ap.offset,
                            ap=[list(e_ap.ap[0]), [TOWN, 2], [1, TOWN]],
                        )
                        nc.vector.tensor_tensor(e_v, e_v, m_rep, OP.mult)
                    for j in range(2):
                        head = 2 * hp + j
                        nc.tensor.matmul(
                            psO[j], Vs[kt][:, head, :], e[:, j * TOWN:(j + 1) * TOWN],
                            start=kt == 0, stop=kt == NKT - 1,
                        )
                ot = p_ot.tile([P, TOWN], bf16, tag="ot")
                for j in range(2):
                    oraw = p_oraw.tile([65, TOWN], f32, tag="oraw")
                    nc.vector.tensor_copy(oraw, psO[j])
                    den = p_small.tile([1, TOWN], f32, tag="sm")
                    nc.vector.tensor_copy(den, oraw[64:65, :])
                    rec = p_small.tile([1, TOWN], f32, tag="sm")
                    nc.vector.reciprocal_approx_fast(rec, den)
                    # broadcast rec across 64 partitions via a DRAM round-trip
                    # (frees PE + DVE of the rank-1 matmul + psum eviction; the
                    # latency hides under the next head-pair's kt loop)
                    recd = p_dram.tile([1, TOWN], f32, tag="recd")
                    nc.sync.dma_start(out=recd, in_=rec)
                    bc = p_bc.tile([64, TOWN], f32, tag="bc")
                    rap = recd[0]
                    nc.sync.dma_start(out=bc, in_=cbass.AP(
                        tensor=rap.tensor, offset=rap.offset,
                        ap=[[0, 64]] + [list(d) for d in rap.ap],
                    ))
                    nc.vector.tensor_tensor(ot[j * 64:(j + 1) * 64, :], oraw[0:64, :], bc, OP.mult)
                OTs.append(ot)
            return OTs

        def proj_residual(w_sb, col_off, n_k, rhs, bias_ap, xs):
            nxs, nxbs, nsqs = [], [], []
            for m in range(NDT):
                ps = ps_mm.tile([P, TOWN], f32, tag="mm")
                for kc in range(n_k):
                    nc.tensor.matmul(
                        ps, w_sb[:, kc, col_off + m * P:col_off + (m + 1) * P],
                        rhs[kc], start=kc == 0, stop=kc == n_k - 1,
                    )
                bcol = p_bias.tile([P, 1], f32, tag="bcol")
                nc.sync.dma_start(out=bcol, in_=bias_ap[m * P:(m + 1) * P, :])
                nx = p_x.tile([P, TOWN], f32, tag="x")
                nc.vector.scalar_tensor_tensor(nx, ps, bcol, xs[m], OP.add, OP.add)
                nxs.append(nx)
                xb, sq = xshadow(nx)
                nxbs.append(xb); nsqs.append(sq)
            return nxs, nxbs, nsqs

        def ffn(w1_ap, w2_ap, b1_ap, b2_ap, hs, xs):
            nxbs, nsqs = [], []
            gs = []
            for m in range(F // P):
                w1m = p_w1.tile([P, NDT, P], bf16, tag="wf1")
                nc.sync.dma_start(
                    out=w1m,
                    in_=w1_ap[:, m * P:(m + 1) * P].rearrange("(kc p) m -> p kc m", p=P),
                )
                ps = ps_mm.tile([P, TOWN], f32, tag="mm")
                for kc in range(NDT):
                    nc.tensor.matmul(
                        ps, w1m[:, kc, :], hs[kc],
                        start=kc == 0, stop=kc == NDT - 1,
                    )
                bcol = p_bias.tile([P, 1], f32, tag="bcol")
                nc.sync.dma_start(out=bcol, in_=b1_ap[m * P:(m + 1) * P, :])
                g = p_g.tile([P, TOWN], bf16, tag="g")
                nc.scalar.activation(g, ps, AF.Gelu, bias=bcol)
                gs.append(g)
            nxs = []
            for m in range(NDT):
                w2m = p_w2.tile([P, F // P, P], bf16, tag="wf2")
                nc.sync.dma_start(
                    out=w2m,
                    in_=w2_ap[:, m * P:(m + 1) * P].rearrange("(kc p) c -> p kc c", p=P),
                )
                ps2 = ps_mm.tile([P, TOWN], f32, tag="mm")
                for kc in range(F // P):
                    nc.tensor.matmul(
                        ps2, w2m[:, kc, :], gs[kc],
                        start=kc == 0, stop=kc == F // P - 1,
                    )
                bcol = p_bias.tile([P, 1], f32, tag="bcol")
                nc.sync.dma_start(out=bcol, in_=b2_ap[m * P:(m + 1) * P, :])
                nx = p_x.tile([P, TOWN], f32, tag="x")
                nc.vector.scalar_tensor_tensor(nx, ps2, bcol, xs[m], OP.add, OP.add)
                nxs.append(nx)
                xb, sq = xshadow(nx)
                nxbs.append(xb); nsqs.append(sq)
            return nxs, nxbs, nsqs

        p_x = ep(tc.tile_pool(name="p_x", bufs=5))
        p_h = ep(tc.tile_pool(name="p_h", bufs=6))
        p_q = ep(tc.tile_pool(name="p_q", bufs=5))
        p_kv = ep(tc.tile_pool(name="p_kv", bufs=4))
        p_v = ep(tc.tile_pool(name="p_v", bufs=8))
        p_kvout = ep(tc.tile_pool(name="p_kvout", bufs=4))
        p_ot = ep(tc.tile_pool(name="p_ot", bufs=4))
        p_e = ep(tc.tile_pool(name="p_e", bufs=2))
        p_oraw = ep(tc.tile_pool(name="p_oraw", bufs=2))
        p_g = ep(tc.tile_pool(name="p_g", bufs=16))
        p_lnsq = ep(tc.tile_pool(name="p_lnsq", bufs=4))
        p_lnac = ep(tc.tile_pool(name="p_lnac", bufs=1))
        p_bc = ep(tc.tile_pool(name="p_bc", bufs=2))
        p_small = ep(tc.tile_pool(name="p_small", bufs=3))
        p_bias = ep(tc.tile_pool(name="p_bias", bufs=4))
        p_eo = ep(tc.tile_pool(name="p_eo", bufs=4))
        p_eob = ep(tc.tile_pool(name="p_eob", bufs=4))
        p_w1 = ep(tc.tile_pool(name="p_w1", bufs=3))
        p_w2 = ep(tc.tile_pool(name="p_w2", bufs=2))
        p_wqkv = ep(tc.tile_pool(name="p_wqkv", bufs=1))
        p_wout = ep(tc.tile_pool(name="p_wout", bufs=2))

        # ================= encoder =================
        xs = []
        xbs, sqs = [], []
        for dt in range(NDT):
            x = p_x.tile([P, TOWN], f32, tag="x")
            nc.sync.dma_start(out=x, in_=x0T[dt])
            xs.append(x)
            xb, sq = xshadow(x)
            xbs.append(xb); sqs.append(sq)
        xst = (xs, xbs, sqs)

        for l in range(n_enc):
            wqkv = load_w(p_wqkv, ew_qkv[l], NDT, 3 * D, "wqkv")
            wout = load_w(p_wout, ew_out[l], NDT, D, "wout")

            hs = layer_norm(xst, bf16, p_h, "h")
            kouts = proj_rank1bias(wqkv, D, hs, eb_qkv_bf[l], D, token_major=False)
            vouts = proj_rank1bias(wqkv, 2 * D, hs, eb_qkv_bf[l], 2 * D, token_major=True)
            boutk, boutv = kv_allgather(kouts, vouts, f"e{l}")
            Qs = proj_fm(wqkv, 0, 4, hs, eb_qkv[l], p_q, "q")
            if l == 0:
                # decoder layer-0 self-attention K/V depends only on dec_in:
                # project + fire its AllGather here (after e0's AG so the
                # collective queue serves e0 first), then build the kNN mask.
                # All of it overlaps the e0 AllGather wait + mask window.
                yt0 = []
                ybt0, sqt0 = [], []
                for dt in range(NDT):
                    y = p_eo.tile([P, TOWN], f32, tag="eof")
                    nc.sync.dma_start(out=y, in_=y0T[dt])
                    yt0.append(y)
                    yb_, sq_ = xshadow(y)
                    ybt0.append(yb_); sqt0.append(sq_)
                hs_d0 = layer_norm((yt0, ybt0, sqt0), bf16, p_eob, "eob")
                wq_d0 = load_w(p_wqkv, dw_saqkv[0], NDT, 3 * D, "wqkv")
                k_d0 = proj_rank1bias(wq_d0, D, hs_d0, db_saqkv_bf[0], D, token_major=False)
                v_d0 = proj_rank1bias(wq_d0, 2 * D, hs_d0, db_saqkv_bf[0], 2 * D, token_major=True)
                d0_bouts = kv_allgather(k_d0, v_d0, "d0")
                hd0_dram = p_dram.tile([NDT, P, TOWN], bf16, tag="hd0")
                for dt in range(NDT):
                    nc.sync.dma_start(out=hd0_dram[dt], in_=hs_d0[dt])
                allow_sb = build_mask()
            pe_warm(36)
            Ks = load_k(boutk)
            Vs = load_v(boutv)
            if dbgkv_t is not None and l == 1:
                for p in range(4):
                    nc.sync.dma_start(out=dbgk_t[p], in_=Ks[p])
                    nc.sync.dma_start(out=dbgko_t[p], in_=kouts[p])
            OTs = attention(Qs, Ks, Vs, allow_sb)
            xst = proj_residual(wout, 0, NDT, OTs, eb_out[l], xst[0])
            dbg(xst[0])
            hs = layer_norm(xst, bf16, p_h, "h")
            xst = ffn(ew_f1[l], ew_f2[l], eb_f1[l], eb_f2[l], hs, xst[0])
            dbg(xst[0])

        pe_warm(12)
        eof = layer_norm(xst, f32, p_eo, "eof")
        eob = []
        for dt in range(NDT):
            nc.sync.dma_start(out=enc_part[dt], in_=eof[dt])
            t = p_eob.tile([P, TOWN], bf16, tag="eob")
            nc.vector.tensor_copy(t, eof[dt])
            eob.append(t)

        # ==== cross-attention K/V: project now, AllGather per decoder layer ====
        bin_cas = []
        with tc.tile_pool(name="p_wca", bufs=1) as p_wca:
            for l in range(n_dec):
                wkv = p_wca.tile([P, NDT, 2 * D], bf16, tag="wcakv")
                nc.sync.dma_start(
                    out=wkv,
                    in_=dw_caqkv[l][:, D:3 * D].rearrange("(kc p) m -> p kc m", p=P),
                )
                kouts = proj_rank1bias(wkv, 0, eob, db_caqkv_bf[l], D, token_major=False)
                vouts = proj_rank1bias(wkv, D, eob, db_caqkv_bf[l], 2 * D, token_major=True)
                bin_k = p_dram.tile([4, P, TOWN], bf16, tag=f"caink{l}")
                bin_v = p_dram.tile([4, P, TOWN], bf16, tag=f"cainv{l}")
                for p in range(4):
                    nc.sync.dma_start(out=bin_k[p], in_=kouts[p])
                    nc.sync.dma_start(out=bin_v[p], in_=vouts[p])
                bin_cas.append((bin_k, bin_v))

        bout_cas = []
        for l in range(n_dec):
            bk = p_dram.tile([2, 4, P, TOWN], bf16, tag=f"caoutk{l}")
            bv = p_dram.tile([2, 4, P, TOWN], bf16, tag=f"caoutv{l}")
            bout_cas.append((bk, bv))
        fire_cross_ag0_early = True

        def fire_cross_ag(l):
            for i in range(2):
                nc.gpsimd.collective_compute(
                    "AllGather", OP.bypass, replica_groups=PAIRS,
                    ins=[bin_cas[l][i][:].opt()], outs=[bout_cas[l][i][:].opt()],
                )

        # ================= decoder =================
        ys = []
        ybs, qsq = [], []
        for dt in range(NDT):
            y = p_x.tile([P, TOWN], f32, tag="x")
            nc.sync.dma_start(out=y, in_=y0T[dt])
            ys.append(y)
            yb, sq_ = xshadow(y)
            ybs.append(yb); qsq.append(sq_)
        yst = (ys, ybs, qsq)

        fire_cross_ag(0)
        for l in range(n_dec):
            wqkv = load_w(p_wqkv, dw_saqkv[l], NDT, 3 * D, "wqkv")
            wout = load_w(p_wout, dw_saout[l], NDT, D, "wout")

            # self-attention (causal)
            if l == 0:
                # K/V AllGather was prefired at program start; reload h
                hs = []
                for dt in range(NDT):
                    h = p_h.tile([P, TOWN], bf16, tag="h")
                    nc.sync.dma_start(out=h, in_=hd0_dram[dt])
                    hs.append(h)
                boutk, boutv = d0_bouts
            else:
                hs = layer_norm(yst, bf16, p_h, "h")
                kouts = proj_rank1bias(wqkv, D, hs, db_saqkv_bf[l], D, token_major=False)
                vouts = proj_rank1bias(wqkv, 2 * D, hs, db_saqkv_bf[l], 2 * D, token_major=True)
                boutk, boutv = kv_allgather(kouts, vouts, f"d{l}")
            Qs = proj_fm(wqkv, 0, 4, hs, db_saqkv[l], p_q, "q")
            pe_warm(36)
            Ks = load_k(boutk)
            Vs = load_v(boutv)
            OTs = attention(Qs, Ks, Vs, causal_sb)
            if l + 1 < n_dec:
                fire_cross_ag(l + 1)
            bout_ca = bout_cas[l]
            yst = proj_residual(wout, 0, NDT, OTs, db_saout[l], yst[0])
            dbg(yst[0])

            # cross-attention (no mask)
            wcaq = load_w(p_wout, dw_caqkv[l][:, 0:D], NDT, D, "wout")
            wcao = load_w(p_wout, dw_caout[l], NDT, D, "wout")
            hs = layer_norm(yst, bf16, p_h, "h")
            Qs = proj_fm(wcaq, 0, 4, hs, db_caqkv[l], p_q, "q")
            Ks = load_k(bout_ca[0])
            Vs = load_v(bout_ca[1])
            OTs = attention(Qs, Ks, Vs, None)
            yst = proj_residual(wcao, 0, NDT, OTs, db_caout[l], yst[0])
            dbg(yst[0])

            # ffn
            hs = layer_norm(yst, bf16, p_h, "h")
            yst = ffn(dw_f1[l], dw_f2[l], db_f1[l], db_f2[l], hs, yst[0])
            dbg(yst[0])

        dof = layer_norm(yst, f32, p_eo, "eof")
        for dt in range(NDT):
            nc.sync.dma_start(out=dec_part[dt], in_=dof[dt])

    nc.compile()
    return nc


def make_in_maps(inputs):
    inp = {k: np.asarray(v) for k, v in inputs.items()}
    f32 = np.float32

    W = {
        "ew_qkv": np.ascontiguousarray(inp["e_qkv_w"].swapaxes(1, 2)).astype(BF16),
        "ew_out": np.ascontiguousarray(inp["e_out_w"].swapaxes(1, 2)).astype(BF16),
        "ew_f1": np.ascontiguousarray(inp["e_ff1_w"].swapaxes(1, 2)).astype(BF16),
        "ew_f2": np.ascontiguousarray(inp["e_ff2_w"].swapaxes(1, 2)).astype(BF16),
        "eb_qkv": inp["e_qkv_b"].astype(f32).reshape(NE, 3 * D, 1),
        "eb_out": inp["e_out_b"].astype(f32).reshape(NE, D, 1),
        "eb_f1": inp["e_ff1_b"].astype(f32).reshape(NE, F, 1),
        "eb_f2": inp["e_ff2_b"].astype(f32).reshape(NE, D, 1),
        "dw_saqkv": np.ascontiguousarray(inp["d_sa_qkv_w"].swapaxes(1, 2)).astype(BF16),
        "db_saqkv": inp["d_sa_qkv_b"].astype(f32).reshape(ND, 3 * D, 1),
        "dw_saout": np.ascontiguousarray(inp["d_sa_out_w"].swapaxes(1, 2)).astype(BF16),
        "db_saout": inp["d_sa_out_b"].astype(f32).reshape(ND, D, 1),
        "dw_caqkv": np.ascontiguousarray(inp["d_ca_qkv_w"].swapaxes(1, 2)).astype(BF16),
        "db_caqkv": inp["d_ca_qkv_b"].astype(f32).reshape(ND, 3 * D, 1),
        "dw_caout": np.ascontiguousarray(inp["d_ca_out_w"].swapaxes(1, 2)).astype(BF16),
        "db_caout": inp["d_ca_out_b"].astype(f32).reshape(ND, D, 1),
        "dw_f1": np.ascontiguousarray(inp["d_ff1_w"].swapaxes(1, 2)).astype(BF16),
        "db_f1": inp["d_ff1_b"].astype(f32).reshape(ND, F, 1),
        "dw_f2": np.ascontiguousarray(inp["d_ff2_w"].swapaxes(1, 2)).astype(BF16),
        "db_f2": inp["d_ff2_b"].astype(f32).reshape(ND, D, 1),
        "eb_qkv_bf": inp["e_qkv_b"].astype(BF16).reshape(NE, 3 * D, 1),
        "db_saqkv_bf": inp["d_sa_qkv_b"].astype(BF16).reshape(ND, 3 * D, 1),
        "db_caqkv_bf": inp["d_ca_qkv_b"].astype(BF16).reshape(ND, 3 * D, 1),
    }

    in_maps = []
    for c in range(NCORE):
        b, half = c // 2, c % 2
        sl = slice(half * TOWN, (half + 1) * TOWN)
        m = dict(W)
        xT = np.ascontiguousarray(inp["enc_in"][b].astype(f32).T[:, sl])
        m["x0T"] = xT.reshape(NDT, P, TOWN)
        yT = np.ascontiguousarray(inp["dec_in"][b].astype(f32).T[:, sl])
        m["y0T"] = yT.reshape(NDT, P, TOWN)
        xyz = inp["enc_xyz"][b].astype(f32)
        n2 = (xyz * xyz).sum(-1, dtype=f32).astype(f32)
        xq2 = (np.float32(2.0) * xyz[sl]).astype(f32)
        m["xq2"] = np.ascontiguousarray(xq2)
        m["xq2row"] = np.ascontiguousarray(xq2.T)
        xkn = np.concatenate([xyz, n2[:, None]], 1).astype(f32)
        m["xkn"] = np.ascontiguousarray(xkn)
        m["xrow"] = np.ascontiguousarray(xkn.T)
        bos = np.full((1, TOWN), 1e30, f32)
        if half == 0:
            bos[0, 0] = NEG
        m["bosrow"] = bos
        qg = np.arange(half * TOWN, (half + 1) * TOWN)
        kg = np.arange(LE)
        m["causal"] = np.ascontiguousarray(
            (kg[:, None] <= qg[None, :]).astype(BF16)
        ).reshape(NKT, P, TOWN)
        in_maps.append(m)
    return in_maps


def assemble(results):
    enc = np.zeros((B, LE, D), np.float32)
    dec = np.zeros((B, LD, D), np.float32)
    for c, r in enumerate(results):
        b, half = c // 2, c % 2
        sl = slice(half * TOWN, (half + 1) * TOWN)
        enc[b, sl, :] = r["enc_part"].reshape(D, TOWN).T
        dec[b, sl, :] = r["dec_part"].reshape(D, TOWN).T
    return enc, dec


def kernel(**inputs):
    from concourse import bass_utils

    if "nc" not in _CACHE:
        _CACHE["nc"] = build()
    nc = _CACHE["nc"]
    in_maps = make_in_maps(inputs)
    enc = dec = None
    for attempt in range(3):
        try:
            res = bass_utils.run_bass_kernel_spmd(
                nc, in_maps, core_ids=list(range(NCORE))
            )
        except Exception:
            if attempt == 2:
                raise
            continue
        enc, dec = assemble(res.results)
        # transient first-execution flakes have shown up as NaN output;
        # the math can never produce NaN, so retry on detection
        if not (np.isnan(enc).any() or np.isnan(dec).any()):
            break
    return enc, dec



# revision 48
# speedup vs baseline: 1.0136x; 1.0136x over previous
"""Trainium2 Bass kernel for nn_EncoderDecoderTransformer (sparse kNN encoder attention).

Sharding: data-parallel over batch (4 batches x 2 cores) with each pair of cores
splitting the sequence dimension (512 tokens each). Per layer, K/V are exchanged
within the pair via AllGather over replica groups [[0,1],[2,3],[4,5],[6,7]].

Layouts (per core):
  - Activations feature-major: x^T stored as 4 tiles (128 dims, 512 own tokens).
  - Q^T/K^T feature-major (head h lives in rows [64*(h%2):...] of ptile h//2).
  - V token-major (128 tokens, 8 heads, 65) with a constant-1 column per head so
    the AV matmul also produces the softmax denominator in psum row 64.
  - Scores computed transposed: S^T = K^T.T @ Q^T  (keys on partitions), exp on
    the Scalar engine, kNN/causal masking as a 0/1 bf16 multiply.
  - kNN mask: s'_qk = 2 x_q.x_k - |x_k|^2 orders like -distance. The 17th
    largest per row (self is always rank 1) is the inclusion threshold; computed
    with the DVE max8/match_replace top-k primitives. Verified to reproduce the
    reference _knn_mask exactly.
"""

import os
import numpy as np
import ml_dtypes

BF16 = ml_dtypes.bfloat16

D, F, H, NE, ND, KNN = 512, 2048, 8, 4, 4, 16
B, LE, LD = 4, 1024, 1024
DH = D // H
NCORE = 8
P = 128
TOWN = 512          # tokens owned per core
NDT = D // P        # 4 feature tiles
NKT = LE // P       # 8 key tiles
NEG = -1e30
EPS = 1e-5
PAIRS = [[0, 1], [2, 3], [4, 5], [6, 7]]

_CACHE = {}


def build(n_enc=NE, n_dec=ND):
    from contextlib import ExitStack

    import concourse.bacc as bacc
    import concourse.tile as tile
    import concourse.mybir as mybir

    f32 = mybir.dt.float32
    bf16 = mybir.dt.bfloat16
    f16 = mybir.dt.float16
    AF = mybir.ActivationFunctionType
    OP = mybir.AluOpType

    import concourse.bass as cbass

    nc = bacc.Bacc("TRN2", target_bir_lowering=False, debug=False, num_devices=NCORE)
    NOGP = not os.environ.get("KQ_GP")  # gpsimd elementwise corrupts data on HW; keep off

    # ---- I/O ----
    def din(name, shape, dt=f32):
        return nc.dram_tensor(name, shape, dt, kind="ExternalInput")

    x0T = din("x0T", [NDT, P, TOWN])
    y0T = din("y0T", [NDT, P, TOWN])
    xq2_d = din("xq2", [TOWN, 3])       # 2*xyz for own tokens
    xq2row_d = din("xq2row", [3, TOWN])  # same, transposed
    xkn_d = din("xkn", [LE, 4])          # [xyz, |xyz|^2] all tokens
    xrow_d = din("xrow", [4, LE])        # same, transposed
    bosrow = din("bosrow", [1, TOWN])
    ident_in = din("ident", [P, P])
    causal_in = din("causal", [NKT, P, TOWN], bf16)

    ew_qkv = din("ew_qkv", [NE, D, 3 * D], bf16)
    ew_out = din("ew_out", [NE, D, D], bf16)
    ew_f1 = din("ew_f1", [NE, D, F], bf16)
    ew_f2 = din("ew_f2", [NE, F, D], bf16)
    eb_qkv = din("eb_qkv", [NE, 3 * D, 1])
    eb_out = din("eb_out", [NE, D, 1])
    eb_f1 = din("eb_f1", [NE, F, 1])
    eb_f2 = din("eb_f2", [NE, D, 1])

    dw_saqkv = din("dw_saqkv", [ND, D, 3 * D], bf16)
    db_saqkv = din("db_saqkv", [ND, 3 * D, 1])
    dw_saout = din("dw_saout", [ND, D, D], bf16)
    db_saout = din("db_saout", [ND, D, 1])
    dw_caqkv = din("dw_caqkv", [ND, D, 3 * D], bf16)
    db_caqkv = din("db_caqkv", [ND, 3 * D, 1])
    dw_caout = din("dw_caout", [ND, D, D], bf16)
    db_caout = din("db_caout", [ND, D, 1])
    dw_f1 = din("dw_f1", [ND, D, F], bf16)
    db_f1 = din("db_f1", [ND, F, 1])
    dw_f2 = din("dw_f2", [ND, F, D], bf16)
    db_f2 = din("db_f2", [ND, D, 1])
    eb_qkv_bf = din("eb_qkv_bf", [NE, 3 * D, 1], bf16)
    db_saqkv_bf = din("db_saqkv_bf", [ND, 3 * D, 1], bf16)
    db_caqkv_bf = din("db_caqkv_bf", [ND, 3 * D, 1], bf16)

    enc_part = nc.dram_tensor("enc_part", [NDT, P, TOWN], f32, kind="ExternalOutput")
    dec_part = nc.dram_tensor("dec_part", [NDT, P, TOWN], f32, kind="ExternalOutput")
    n_dbg = int(os.environ.get("KQ_DEBUG", "0"))
    dbg_t = None
    if n_dbg:
        dbg_t = nc.dram_tensor("dbg", [n_dbg, NDT, P, TOWN], f32, kind="ExternalOutput")
    dbg_i = [0]
    dbgkv_t = None
    if os.environ.get("KQ_DEBUG_KV"):
        dbgkv_t = nc.dram_tensor("dbgkv", [2, 2, 4, P, TOWN], bf16, kind="ExternalOutput")
        dbgk_t = nc.dram_tensor("dbgk", [4, P, 2, TOWN], bf16, kind="ExternalOutput")
        dbgko_t = nc.dram_tensor("dbgko", [4, P, TOWN], bf16, kind="ExternalOutput")
    dbga_t = None
    if os.environ.get("KQ_DEBUG_ALLOW"):
        dbga_t = nc.dram_tensor("dbga", [NKT, P, TOWN], bf16, kind="ExternalOutput")
        dbgs_t = nc.dram_tensor("dbgs", [NKT, P, TOWN], f32, kind="ExternalOutput")
        dbgt_t = nc.dram_tensor("dbgt", [1, TOWN], f32, kind="ExternalOutput")

    with tile.TileContext(nc) as tc, ExitStack() as ctx:
        ep = ctx.enter_context

        pc = ep(tc.tile_pool(name="pc", bufs=1))
        p_allow = ep(tc.tile_pool(name="p_allow", bufs=8))
        p_causal = ep(tc.tile_pool(name="p_causal", bufs=8))
        # PSUM budget (8 banks): ps_s pair-tiles [P,1024] x2 = 4, ps_o [65,512] x2 = 2,
        # ps_mm [*,512] x2 = 2.
        ps_s = ep(tc.tile_pool(name="ps_s", bufs=2, space="PSUM"))
        ps_o = ep(tc.tile_pool(name="ps_o", bufs=2, space="PSUM"))
        ps_mm = ep(tc.tile_pool(name="ps_mm", bufs=2, space="PSUM"))
        p_dram = ep(tc.tile_pool(name="p_dram", bufs=2, space="DRAM"))

        # ---- constants ----
        ones_col = pc.tile([P, 1], f32)          # LN partition sums (lhsT)
        nc.vector.memset(ones_col, 1.0)
        ones_row = pc.tile([1, P], f32)          # broadcast lhsT (K=1)
        nc.vector.memset(ones_row, 1.0)
        ones_row512 = pc.tile([1, TOWN], f32)    # rhs for row-rank1 bias matmuls
        nc.vector.memset(ones_row512, 1.0)
        ones_col_bf = pc.tile([P, 1], bf16)
        nc.vector.memset(ones_col_bf, 1.0)
        ones_row_bf = pc.tile([1, P], bf16)
        nc.vector.memset(ones_row_bf, 1.0)
        ones_row512_bf = pc.tile([1, TOWN], bf16)
        nc.vector.memset(ones_row512_bf, 1.0)
        ones_row_f16 = pc.tile([1, P], f16)
        nc.vector.memset(ones_row_f16, 1.0)
        dummy_w = pc.tile([P, P], bf16)
        nc.vector.memset(dummy_w, 0.0)
        dummy_x = pc.tile([P, TOWN], bf16)
        nc.vector.memset(dummy_x, 0.0)
        eps_sb = pc.tile([1, 1], f32)
        nc.vector.memset(eps_sb, EPS)

        bos_sb = pc.tile([1, TOWN], f32)
        nc.sync.dma_start(out=bos_sb, in_=bosrow[:, :])
        ident_sb = pc.tile([P, P], f32)
        nc.sync.dma_start(out=ident_sb, in_=ident_in[:, :])

        causal_sb = []
        for kt in range(NKT):
            t = p_causal.tile([P, TOWN], bf16, tag="causal")
            nc.sync.dma_start(out=t, in_=causal_in[kt])
            causal_sb.append(t)

        def dbg(ts):
            if dbg_t is None or dbg_i[0] >= n_dbg:
                return
            for dt in range(NDT):
                nc.sync.dma_start(out=dbg_t[dbg_i[0], dt], in_=ts[dt])
            dbg_i[0] += 1

        def pe_warm(n):
            # filler matmuls emitted into PE-idle windows (AG waits, mask
            # phase) so the HAM clock gate stays at full rate; never read
            psD = ps_mm.tile([P, TOWN], f32, tag="mm")
            for _ in range(n):
                nc.tensor.matmul(psD, dummy_w, dummy_x, start=True, stop=True)

        def build_mask():

            # s'_qk = 2 x_q . x_k - |x_k|^2 computed with IEEE-exact fp32 DVE ops
            # (the PE fp32 matmul is not exact fp32 and flips kNN boundary choices).
            # Both layouts use the same per-element op chain => bit-identical values.
            def bcast_rows(dram_row_ap, pool, n_free, tag):
                # (n_free,) DRAM row -> (P, n_free) SBUF tile, replicated across partitions
                t = pool.tile([P, n_free], f32, tag=tag)
                src_ap = cbass.AP(
                    tensor=dram_row_ap.tensor, offset=dram_row_ap.offset,
                    ap=[[0, P]] + list(dram_row_ap.ap),
                )
                nc.sync.dma_start(out=t, in_=src_ap)
                return t

            allow_sb = []
            with tc.tile_pool(name="p_mask", bufs=2) as p_mask, \
                 tc.tile_pool(name="p_mbc", bufs=1) as p_mbc, \
                 tc.tile_pool(name="p_m8", bufs=8) as p_m8:
                bcx = []
                for c in range(4):
                    t = bcast_rows(xrow_d[c], p_mbc, LE, tag=f"bcx{c}")
                    bcx.append(t)
                psT = ps_mm.tile([1, TOWN], f32, tag="mm")
                for qt in range(4):
                    xqc = p_m8.tile([P, 3], f32, tag="xqc")
                    nc.sync.dma_start(out=xqc, in_=xq2_d[qt * P:(qt + 1) * P, :])
                    s0 = p_mask.tile([P, LE], f32, tag="s")
                    nc.vector.tensor_scalar(s0, bcx[0], xqc[:, 0:1], None, op0=OP.mult)
                    s1 = p_mask.tile([P, LE], f32, tag="s")
                    nc.vector.scalar_tensor_tensor(s1, bcx[1], xqc[:, 1:2], s0, OP.mult, OP.add)
                    s2 = p_mask.tile([P, LE], f32, tag="s")
                    nc.vector.scalar_tensor_tensor(s2, bcx[2], xqc[:, 2:3], s1, OP.mult, OP.add)
                    s3 = p_mask.tile([P, LE], f32, tag="s")
                    nc.vector.tensor_tensor(s3, s2, bcx[3], OP.subtract)
                    m8 = p_m8.tile([P, 8], f32, tag="m8")
                    nc.vector.max(m8, s3)
                    s4 = p_mask.tile([P, LE], f32, tag="s")
                    nc.vector.match_replace(s4, m8, s3, NEG)
                    m8b = p_m8.tile([P, 8], f32, tag="m8")
                    nc.vector.max(m8b, s4)
                    s5 = p_mask.tile([P, LE], f32, tag="s")
                    nc.vector.match_replace(s5, m8b, s4, NEG)
                    m8c = p_m8.tile([P, 8], f32, tag="m8")
                    nc.vector.max(m8c, s5)
                    # rank-17 value (16 NN + self) is the inclusion threshold;
                    # PE-transpose the per-query column into one PSUM row
                    # (replaces a 512-descriptor DRAM gather that stalled ~50us)
                    nc.tensor.matmul(
                        psT[0:1, qt * P:(qt + 1) * P], m8c[:, 0:1], ident_sb,
                        is_transpose=True, start=qt == 0, stop=qt == 3,
                        skip_group_check=True,
                    )
                t_row = pc.tile([1, TOWN], f32)
                nc.vector.tensor_copy(t_row, psT)
                t2 = pc.tile([1, TOWN], f32)
                nc.vector.tensor_tensor(t2, t_row, bos_sb, OP.min)
                psBC = ps_mm.tile([P, TOWN], f32, tag="mm")
                nc.tensor.matmul(psBC, ones_row, t2, start=True, stop=True)
                t_bc = pc.tile([P, TOWN], f32)
                nc.vector.tensor_copy(t_bc, psBC)
                bq = []
                for c in range(3):
                    t = bcast_rows(xq2row_d[c], p_mbc, TOWN, tag=f"bq{c}")
                    bq.append(t)
                for kt in range(NKT):
                    xkc = p_m8.tile([P, 4], f32, tag="xkc")
                    nc.sync.dma_start(out=xkc, in_=xkn_d[kt * P:(kt + 1) * P, :])
                    u0 = p_mask.tile([P, TOWN], f32, tag="st")
                    nc.vector.tensor_scalar(u0, bq[0], xkc[:, 0:1], None, op0=OP.mult)
                    u1 = p_mask.tile([P, TOWN], f32, tag="st")
                    nc.vector.scalar_tensor_tensor(u1, bq[1], xkc[:, 1:2], u0, OP.mult, OP.add)
                    u2 = p_mask.tile([P, TOWN], f32, tag="st")
                    nc.vector.scalar_tensor_tensor(u2, bq[2], xkc[:, 2:3], u1, OP.mult, OP.add)
                    u3 = p_mask.tile([P, TOWN], f32, tag="st")
                    nc.vector.tensor_scalar(u3, u2, xkc[:, 3:4], None, op0=OP.subtract)
                    al = p_allow.tile([P, TOWN], bf16, tag="allow")
                    nc.vector.tensor_tensor(al, u3, t_bc, OP.is_ge)
                    if kt == 0:
                        # BOS key allowed for all q; emitted here (not after the
                        # loop) so attention on kt=0 can start immediately
                        nc.vector.memset(al[0:1, :], 1.0)
                    if dbga_t is not None:
                        nc.sync.dma_start(out=dbgs_t[kt], in_=u3)
                    allow_sb.append(al)
                if dbga_t is not None:
                    nc.sync.dma_start(out=dbgt_t[:, :], in_=t2)
                    for kt in range(NKT):
                        nc.sync.dma_start(out=dbga_t[kt], in_=allow_sb[kt])
            return allow_sb



        # ================= helpers =================
        def load_w(pool, dram_ap, kchunks, cols, tag):
            t = pool.tile([P, kchunks, cols], bf16, tag=tag)
            nc.sync.dma_start(
                out=t, in_=dram_ap.rearrange("(kc p) m -> p kc m", p=P)
            )
            return t

        def xshadow(x):
            # bf16 shadow + square, emitted at x-production time so the next
            # layer_norm starts directly with its reduction matmuls
            xb = p_lnsq.tile([P, TOWN], bf16, tag="lnxb")
            nc.vector.tensor_copy(xb, x)
            sq = p_lnsq.tile([P, TOWN], bf16, tag="lnsq")
            nc.scalar.activation(sq, x, AF.Square)
            return xb, sq

        def layer_norm(xst, out_dt, out_pool, out_tag):
            xs, xbs, sqs = xst
            ps_mean = ps_mm.tile([1, TOWN], f32, tag="mm")
            for dt in range(NDT):
                nc.tensor.matmul(ps_mean, ones_col_bf, xbs[dt], start=dt == 0, stop=dt == 3)
            ps_sq = ps_mm.tile([1, TOWN], f32, tag="mm")
            for dt in range(NDT):
                nc.tensor.matmul(ps_sq, ones_col_bf, sqs[dt], start=dt == 0, stop=dt == 3)
            mu16 = p_small.tile([1, TOWN], f16, tag="smf16")
            nc.scalar.mul(mu16, ps_mean, 1.0 / D)
            musq = p_small.tile([1, TOWN], f32, tag="sm")
            nc.scalar.activation(musq, ps_mean, AF.Square, scale=1.0 / D)
            var = p_small.tile([1, TOWN], f32, tag="sm")
            nc.vector.scalar_tensor_tensor(var, ps_sq, 1.0 / D, musq, OP.mult, OP.subtract)
            lnv = p_small.tile([1, TOWN], f32, tag="sm")
            nc.scalar.activation(lnv, var, AF.Ln, bias=eps_sb)
            rstd = p_small.tile([1, TOWN], f16, tag="smf16")
            nc.scalar.activation(rstd, lnv, AF.Exp, scale=-0.5)
            # h = (x - mu)*rstd via f16 rank-1 broadcasts of mu and rstd
            ps_a = ps_mm.tile([P, TOWN], f32, tag="mm")
            nc.tensor.matmul(ps_a, ones_row_f16, rstd, start=True, stop=True)
            ps_c = ps_mm.tile([P, TOWN], f32, tag="mm")
            nc.tensor.matmul(ps_c, ones_row_f16, mu16, start=True, stop=True)
            a_sb = p_lnac.tile([P, TOWN], f32, tag="lna")
            nc.vector.tensor_copy(a_sb, ps_a)
            c_sb = p_lnac.tile([P, TOWN], f32, tag="lnc")
            nc.vector.tensor_copy(c_sb, ps_c)
            hs = []
            for dt in range(NDT):
                h = out_pool.tile([P, TOWN], out_dt, tag=out_tag)
                nc.vector.tensor_tensor(h, xs[dt], c_sb, OP.subtract)
                (nc.vector if NOGP else nc.gpsimd).tensor_tensor(h, h, a_sb, OP.mult)
                hs.append(h)
            return hs

        def proj_fm(w_sb, col_off, n_m, rhs, bias_ap, out_pool, out_tag, out_dt=bf16):
            """Feature-major projection; per-partition bias applied on eviction."""
            outs = []
            nk = len(rhs)
            for m in range(n_m):
                ps = ps_mm.tile([P, TOWN], f32, tag="mm")
                for kc in range(nk):
                    nc.tensor.matmul(
                        ps, w_sb[:, kc, col_off + m * P:col_off + (m + 1) * P],
                        rhs[kc], start=kc == 0, stop=kc == nk - 1,
                    )
                bcol = p_bias.tile([P, 1], f32, tag="bcol")
                nc.sync.dma_start(out=bcol, in_=bias_ap[col_off + m * P:col_off + (m + 1) * P, :])
                o = out_pool.tile([P, TOWN], out_dt, tag=out_tag)
                nc.vector.tensor_scalar(o, ps, bcol, None, op0=OP.add)
                outs.append(o)
            return outs

        def proj_rank1bias(w_sb, col_off, rhs, bias_ap, bias_off, token_major):
            """K^T (feature-major) or V (token-major) projection with the bias
            folded in as a rank-1 bf16 matmul; evicted compact bf16 for the AG bounce."""
            outs = []
            brow512 = None
            if token_major:
                brow512 = p_bias.tile([1, TOWN], bf16, tag="brow512")
                nc.sync.dma_start(
                    out=brow512,
                    in_=bias_ap[bias_off:bias_off + D, :].rearrange("a b -> b a"),
                )
            for m in range(4):
                ps = ps_mm.tile([P, TOWN], f32, tag="mm")
                if token_major:
                    for kc in range(4):
                        nc.tensor.matmul(
                            ps, rhs[kc][:, m * P:(m + 1) * P],
                            w_sb[:, kc, col_off:col_off + D],
                            start=kc == 0, stop=False,
                        )
                    nc.tensor.matmul(ps, ones_row_bf, brow512, start=False, stop=True)
                else:
                    for kc in range(4):
                        nc.tensor.matmul(
                            ps, w_sb[:, kc, col_off + m * P:col_off + (m + 1) * P],
                            rhs[kc], start=kc == 0, stop=False,
                        )
                    brow = p_bias.tile([1, P], bf16, tag="brow")
                    nc.sync.dma_start(
                        out=brow,
                        in_=bias_ap[bias_off + m * P:bias_off + (m + 1) * P, :].rearrange("a b -> b a"),
                    )
                    nc.tensor.matmul(ps, brow, ones_row512_bf, start=False, stop=True)
                o = p_kvout.tile([P, TOWN], bf16, tag="kvout")
                nc.vector.tensor_copy(o, ps)
                outs.append(o)
            return outs

        def kv_allgather(k_outs, v_outs, uniq):
            # one DRAM buffer set per layer: a pool-recycled buffer could be
            # rewritten while the pair-peer's collective still reads it
            # (cross-core hazard invisible to Tile's per-core dep tracking).
            # K and V go in separate AllGathers so attention (which needs K
            # first) can start while V is still in flight.
            bin_k = p_dram.tile([4, P, TOWN], bf16, tag=f"agink{uniq}")
            for p in range(4):
                nc.sync.dma_start(out=bin_k[p], in_=k_outs[p])
            bout_k = p_dram.tile([2, 4, P, TOWN], bf16, tag=f"agoutk{uniq}")
            nc.gpsimd.collective_compute(
                "AllGather", OP.bypass, replica_groups=PAIRS,
                ins=[bin_k[:].opt()], outs=[bout_k[:].opt()],
            )
            bin_v = p_dram.tile([4, P, TOWN], bf16, tag=f"aginv{uniq}")
            for p in range(4):
                nc.sync.dma_start(out=bin_v[p], in_=v_outs[p])
            bout_v = p_dram.tile([2, 4, P, TOWN], bf16, tag=f"agoutv{uniq}")
            nc.gpsimd.collective_compute(
                "AllGather", OP.bypass, replica_groups=PAIRS,
                ins=[bin_v[:].opt()], outs=[bout_v[:].opt()],
            )
            return (bout_k, bout_v)

        def load_k(bout):
            Ks = []
            for p in range(4):
                kt = p_kv.tile([P, 2, TOWN], bf16, tag="ksb")
                for r in range(2):
                    nc.sync.dma_start(out=kt[:, r, :], in_=bout[r, p])
                Ks.append(kt)
            return Ks

        def load_v(bout):
            Vs = []
            for r in range(2):
                for tt in range(4):
                    vt = p_v.tile([P, H, 65], bf16, tag="vsb")
                    nc.sync.dma_start(
                        out=vt[:, :, 0:64],
                        in_=bout[r, tt].rearrange("p (h d) -> p h d", h=H),
                    )
                    (nc.vector if NOGP else nc.gpsimd).memset(vt[:, :, 64:65], 1.0)
                    Vs.append(vt)
            return Vs

        def attention(Qs, Ks, Vs, mask_tiles):
            # Scores for the head pair land in one 2-bank PSUM tile [P, 2*TOWN]
            # (j=0 cols 0:TOWN, j=1 cols TOWN:2*TOWN) so exp and the mask
            # multiply run once per (hp, kt) at N=1024 instead of twice at
            # N=512 — the serial exp->mask chain was the attention rate
            # limiter. psO is evicted to SBUF with a single copy so the two
            # PSUM banks recycle quickly.
            OTs = []
            for hp in range(4):
                psO = []
                for _j in range(2):
                    psO_t = ps_o.tile([65, TOWN], f32, tag="pso")
                    psO.append(psO_t)
                for kt in range(NKT):
                    r, c = kt // 4, kt % 4
                    psP = ps_s.tile([P, 2 * TOWN], f32, tag="pss")
                    for j in range(2):
                        rows = slice(j * 64, (j + 1) * 64)
                        nc.tensor.matmul(
                            psP[:, j * TOWN:(j + 1) * TOWN],
                            Ks[hp][rows, r, c * P:(c + 1) * P], Qs[hp][rows, :],
                            start=True, stop=True,
                        )
                    e = p_e.tile([P, 2 * TOWN], bf16, tag="e")
                    nc.scalar.activation(e, psP, AF.Exp, scale=0.125)
                    if mask_tiles is not None:
                        # one multiply over both halves; the [P, TOWN] mask is
                        # repeated via a stride-0 middle AP dim
                        m_ap = mask_tiles[kt][:, :]
                        m_rep = cbass.AP(
                            tensor=m_ap.tensor, offset=m_ap.offset,
                            ap=[list(m_ap.ap[0]), [0, 2]] + [list(d) for d in m_ap.ap[1:]],
                        )
                        e_ap = e[:, :]
                        e_v = cbass.AP(
                            tensor=e_ap.tensor, offset=e_ap.offset,
                            ap=[list(e_ap.ap[0]), [TOWN, 2], [1, TOWN]],
                        )
                        nc.vector.tensor_tensor(e_v, e_v, m_rep, OP.mult)
                    for j in range(2):
                        head = 2 * hp + j
                        nc.tensor.matmul(
                            psO[j], Vs[kt][:, head, :], e[:, j * TOWN:(j + 1) * TOWN],
                            start=kt == 0, stop=kt == NKT - 1,
                        )
                ot = p_ot.tile([P, TOWN], bf16, tag="ot")
                for j in range(2):
                    oraw = p_oraw.tile([65, TOWN], f32, tag="oraw")
                    nc.vector.tensor_copy(oraw, psO[j])
                    den = p_small.tile([1, TOWN], f32, tag="sm")
                    nc.vector.tensor_copy(den, oraw[64:65, :])
                    rec = p_small.tile([1, TOWN], f32, tag="sm")
                    nc.vector.reciprocal_approx_fast(rec, den)
                    # broadcast rec across 64 partitions via a DRAM round-trip
                    # (frees PE + DVE of the rank-1 matmul + psum eviction; the
                    # latency hides under the next head-pair's kt loop)
                    recd = p_dram.tile([1, TOWN], f32, tag="recd")
                    nc.sync.dma_start(out=recd, in_=rec)
                    bc = p_bc.tile([64, TOWN], f32, tag="bc")
                    rap = recd[0]
                    nc.sync.dma_start(out=bc, in_=cbass.AP(
                        tensor=rap.tensor, offset=rap.offset,
                        ap=[[0, 64]] + [list(d) for d in rap.ap],
                    ))
                    nc.vector.tensor_tensor(ot[j * 64:(j + 1) * 64, :], oraw[0:64, :], bc, OP.mult)
                OTs.append(ot)
            return OTs

        def proj_residual(w_sb, col_off, n_k, rhs, bias_ap, xs):
            nxs, nxbs, nsqs = [], [], []
            for m in range(NDT):
                ps = ps_mm.tile([P, TOWN], f32, tag="mm")
                for kc in range(n_k):
                    nc.tensor.matmul(
                        ps, w_sb[:, kc, col_off + m * P:col_off + (m + 1) * P],
                        rhs[kc], start=kc == 0, stop=kc == n_k - 1,
                    )
                bcol = p_bias.tile([P, 1], f32, tag="bcol")
                nc.sync.dma_start(out=bcol, in_=bias_ap[m * P:(m + 1) * P, :])
                nx = p_x.tile([P, TOWN], f32, tag="x")
                nc.vector.scalar_tensor_tensor(nx, ps, bcol, xs[m], OP.add, OP.add)
                nxs.append(nx)
                xb, sq = xshadow(nx)
                nxbs.append(xb); nsqs.append(sq)
            return nxs, nxbs, nsqs

        def ffn(w1_ap, w2_ap, b1_ap, b2_ap, hs, xs):
            nxbs, nsqs = [], []
            gs = []
            for m in range(F // P):
                w1m = p_w1.tile([P, NDT, P], bf16, tag="wf1")
                nc.sync.dma_start(
                    out=w1m,
                    in_=w1_ap[:, m * P:(m + 1) * P].rearrange("(kc p) m -> p kc m", p=P),
                )
                ps = ps_mm.tile([P, TOWN], f32, tag="mm")
                for kc in range(NDT):
                    nc.tensor.matmul(
                        ps, w1m[:, kc, :], hs[kc],
                        start=kc == 0, stop=kc == NDT - 1,
                    )
                bcol = p_bias.tile([P, 1], f32, tag="bcol")
                nc.sync.dma_start(out=bcol, in_=b1_ap[m * P:(m + 1) * P, :])
                g = p_g.tile([P, TOWN], bf16, tag="g")
                nc.scalar.activation(g, ps, AF.Gelu, bias=bcol)
                gs.append(g)
            nxs = []
            for m in range(NDT):
                w2m = p_w2.tile([P, F // P, P], bf16, tag="wf2")
                nc.sync.dma_start(
                    out=w2m,
                    in_=w2_ap[:, m * P:(m + 1) * P].rearrange("(kc p) c -> p kc c", p=P),
                )
                ps2 = ps_mm.tile([P, TOWN], f32, tag="mm")
                for kc in range(F // P):
                    nc.tensor.matmul(
                        ps2, w2m[:, kc, :], gs[kc],
                        start=kc == 0, stop=kc == F // P - 1,
                    )
                bcol = p_bias.tile([P, 1], f32, tag="bcol")
                nc.sync.dma_start(out=bcol, in_=b2_ap[m * P:(m + 1) * P, :])
                nx = p_x.tile([P, TOWN], f32, tag="x")
                nc.vector.scalar_tensor_tensor(nx, ps2, bcol, xs[m], OP.add, OP.add)
                nxs.append(nx)
                xb, sq = xshadow(nx)
                nxbs.append(xb); nsqs.append(sq)
            return nxs, nxbs, nsqs

        p_x = ep(tc.tile_pool(name="p_x", bufs=5))
        p_h = ep(tc.tile_pool(name="p_h", bufs=6))
        p_q = ep(tc.tile_pool(name="p_q", bufs=5))
        p_kv = ep(tc.tile_pool(name="p_kv", bufs=4))
        p_v = ep(tc.tile_pool(name="p_v", bufs=8))
        p_kvout = ep(tc.tile_pool(name="p_kvout", bufs=4))
        p_ot = ep(tc.tile_pool(name="p_ot", bufs=4))
        p_e = ep(tc.tile_pool(name="p_e", bufs=2))
        p_oraw = ep(tc.tile_pool(name="p_oraw", bufs=2))
        p_g = ep(tc.tile_pool(name="p_g", bufs=16))
        p_lnsq = ep(tc.tile_pool(name="p_lnsq", bufs=4))
        p_lnac = ep(tc.tile_pool(name="p_lnac", bufs=1))
        p_bc = ep(tc.tile_pool(name="p_bc", bufs=2))
        p_small = ep(tc.tile_pool(name="p_small", bufs=3))
        p_bias = ep(tc.tile_pool(name="p_bias", bufs=4))
        p_eo = ep(tc.tile_pool(name="p_eo", bufs=4))
        p_eob = ep(tc.tile_pool(name="p_eob", bufs=4))
        p_w1 = ep(tc.tile_pool(name="p_w1", bufs=3))
        p_w2 = ep(tc.tile_pool(name="p_w2", bufs=2))
        p_wqkv = ep(tc.tile_pool(name="p_wqkv", bufs=1))
        p_wout = ep(tc.tile_pool(name="p_wout", bufs=2))

        # ================= encoder =================
        xs = []
        xbs, sqs = [], []
        for dt in range(NDT):
            x = p_x.tile([P, TOWN], f32, tag="x")
            nc.sync.dma_start(out=x, in_=x0T[dt])
            xs.append(x)
            xb, sq = xshadow(x)
            xbs.append(xb); sqs.append(sq)
        xst = (xs, xbs, sqs)

        for l in range(n_enc):
            wqkv = load_w(p_wqkv, ew_qkv[l], NDT, 3 * D, "wqkv")
            wout = load_w(p_wout, ew_out[l], NDT, D, "wout")

            hs = layer_norm(xst, bf16, p_h, "h")
            kouts = proj_rank1bias(wqkv, D, hs, eb_qkv_bf[l], D, token_major=False)
            vouts = proj_rank1bias(wqkv, 2 * D, hs, eb_qkv_bf[l], 2 * D, token_major=True)
            boutk, boutv = kv_allgather(kouts, vouts, f"e{l}")
            Qs = proj_fm(wqkv, 0, 4, hs, eb_qkv[l], p_q, "q")
            if l == 0:
                # decoder layer-0 self-attention K/V depends only on dec_in:
                # project + fire its AllGather here (after e0's AG so the
                # collective queue serves e0 first), then build the kNN mask.
                # All of it overlaps the e0 AllGather wait + mask window.
                yt0 = []
                ybt0, sqt0 = [], []
                for dt in range(NDT):
                    y = p_eo.tile([P, TOWN], f32, tag="eof")
                    nc.sync.dma_start(out=y, in_=y0T[dt])
                    yt0.append(y)
                    yb_, sq_ = xshadow(y)
                    ybt0.append(yb_); sqt0.append(sq_)
                hs_d0 = layer_norm((yt0, ybt0, sqt0), bf16, p_eob, "eob")
                wq_d0 = load_w(p_wqkv, dw_saqkv[0], NDT, 3 * D, "wqkv")
                k_d0 = proj_rank1bias(wq_d0, D, hs_d0, db_saqkv_bf[0], D, token_major=False)
                v_d0 = proj_rank1bias(wq_d0, 2 * D, hs_d0, db_saqkv_bf[0], 2 * D, token_major=True)
                d0_bouts = kv_allgather(k_d0, v_d0, "d0")
                hd0_dram = p_dram.tile([NDT, P, TOWN], bf16, tag="hd0")
                for dt in range(NDT):
                    nc.sync.dma_start(out=hd0_dram[dt], in_=hs_d0[dt])
                allow_sb = build_mask()
            pe_warm(36)
            Ks = load_k(boutk)
            Vs = load_v(boutv)
            if dbgkv_t is not None and l == 1:
                for p in range(4):
                    nc.sync.dma_start(out=dbgk_t[p], in_=Ks[p])
                    nc.sync.dma_start(out=dbgko_t[p], in_=kouts[p])
            OTs = attention(Qs, Ks, Vs, allow_sb)
            xst = proj_residual(wout, 0, NDT, OTs, eb_out[l], xst[0])
            dbg(xst[0])
            hs = layer_norm(xst, bf16, p_h, "h")
            xst = ffn(ew_f1[l], ew_f2[l], eb_f1[l], eb_f2[l], hs, xst[0])
            dbg(xst[0])

        pe_warm(12)
        eof = layer_norm(xst, f32, p_eo, "eof")
        eob = []
        for dt in range(NDT):
            nc.sync.dma_start(out=enc_part[dt], in_=eof[dt])
            t = p_eob.tile([P, TOWN], bf16, tag="eob")
            nc.vector.tensor_copy(t, eof[dt])
            eob.append(t)

        # ==== cross-attention K/V: project now, AllGather per decoder layer ====
        bin_cas = []
        with tc.tile_pool(name="p_wca", bufs=1) as p_wca:
            for l in range(n_dec):
                wkv = p_wca.tile([P, NDT, 2 * D], bf16, tag="wcakv")
                nc.sync.dma_start(
                    out=wkv,
                    in_=dw_caqkv[l][:, D:3 * D].rearrange("(kc p) m -> p kc m", p=P),
                )
                kouts = proj_rank1bias(wkv, 0, eob, db_caqkv_bf[l], D, token_major=False)
                vouts = proj_rank1bias(wkv, D, eob, db_caqkv_bf[l], 2 * D, token_major=True)
                bin_k = p_dram.tile([4, P, TOWN], bf16, tag=f"caink{l}")
                bin_v = p_dram.tile([4, P, TOWN], bf16, tag=f"cainv{l}")
                for p in range(4):
                    nc.sync.dma_start(out=bin_k[p], in_=kouts[p])
                    nc.sync.dma_start(out=bin_v[p], in_=vouts[p])
                bin_cas.append((bin_k, bin_v))

        bout_cas = []
        for l in range(n_dec):
            bk = p_dram.tile([2, 4, P, TOWN], bf16, tag=f"caoutk{l}")
            bv = p_dram.tile([2, 4, P, TOWN], bf16, tag=f"caoutv{l}")
            bout_cas.append((bk, bv))
        fire_cross_ag0_early = True

        def fire_cross_ag(l):
            for i in range(2):
                nc.gpsimd.collective_compute(
                    "AllGather", OP.bypass, replica_groups=PAIRS,
                    ins=[bin_cas[l][i][:].opt()], outs=[bout_cas[l][i][:].opt()],
                )

        # ================= decoder =================
        ys = []
        ybs, qsq = [], []
        for dt in range(NDT):
            y = p_x.tile([P, TOWN], f32, tag="x")
            nc.sync.dma_start(out=y, in_=y0T[dt])
            ys.append(y)
            yb, sq_ = xshadow(y)
            ybs.append(yb); qsq.append(sq_)
        yst = (ys, ybs, qsq)

        fire_cross_ag(0)
        for l in range(n_dec):
            wqkv = load_w(p_wqkv, dw_saqkv[l], NDT, 3 * D, "wqkv")
            wout = load_w(p_wout, dw_saout[l], NDT, D, "wout")

            # self-attention (causal)
            if l == 0:
                # K/V AllGather was prefired at program start; reload h
                hs = []
                for dt in range(NDT):
                    h = p_h.tile([P, TOWN], bf16, tag="h")
                    nc.sync.dma_start(out=h, in_=hd0_dram[dt])
                    hs.append(h)
                boutk, boutv = d0_bouts
            else:
                hs = layer_norm(yst, bf16, p_h, "h")
                kouts = proj_rank1bias(wqkv, D, hs, db_saqkv_bf[l], D, token_major=False)
                vouts = proj_rank1bias(wqkv, 2 * D, hs, db_saqkv_bf[l], 2 * D, token_major=True)
                boutk, boutv = kv_allgather(kouts, vouts, f"d{l}")
            Qs = proj_fm(wqkv, 0, 4, hs, db_saqkv[l], p_q, "q")
            pe_warm(36)
            Ks = load_k(boutk)
            Vs = load_v(boutv)
            OTs = attention(Qs, Ks, Vs, causal_sb)
            if l + 1 < n_dec:
                fire_cross_ag(l + 1)
            bout_ca = bout_cas[l]
            yst = proj_residual(wout, 0, NDT, OTs, db_saout[l], yst[0])
            dbg(yst[0])

            # cross-attention (no mask)
            wcaq = load_w(p_wout, dw_caqkv[l][:, 0:D], NDT, D, "wout")
            wcao = load_w(p_wout, dw_caout[l], NDT, D, "wout")
            hs = layer_norm(yst, bf16, p_h, "h")
            Qs = proj_fm(wcaq, 0, 4, hs, db_caqkv[l], p_q, "q")
            Ks = load_k(bout_ca[0])
            Vs = load_v(bout_ca[1])
            OTs = attention(Qs, Ks, Vs, None)
            yst = proj_residual(wcao, 0, NDT, OTs, db_caout[l], yst[0])
            dbg(yst[0])

            # ffn
            hs = layer_norm(yst, bf16, p_h, "h")
            yst = ffn(dw_f1[l], dw_f2[l], db_f1[l], db_f2[l], hs, yst[0])
            dbg(yst[0])

        dof = layer_norm(yst, f32, p_eo, "eof")
        for dt in range(NDT):
            nc.sync.dma_start(out=dec_part[dt], in_=dof[dt])

    nc.compile()
    return nc


def make_in_maps(inputs):
    inp = {k: np.asarray(v) for k, v in inputs.items()}
    f32 = np.float32

    W = {
        "ew_qkv": np.ascontiguousarray(inp["e_qkv_w"].swapaxes(1, 2)).astype(BF16),
        "ew_out": np.ascontiguousarray(inp["e_out_w"].swapaxes(1, 2)).astype(BF16),
        "ew_f1": np.ascontiguousarray(inp["e_ff1_w"].swapaxes(1, 2)).astype(BF16),
        "ew_f2": np.ascontiguousarray(inp["e_ff2_w"].swapaxes(1, 2)).astype(BF16),
        "eb_qkv": inp["e_qkv_b"].astype(f32).reshape(NE, 3 * D, 1),
        "eb_out": inp["e_out_b"].astype(f32).reshape(NE, D, 1),
        "eb_f1": inp["e_ff1_b"].astype(f32).reshape(NE, F, 1),
        "eb_f2": inp["e_ff2_b"].astype(f32).reshape(NE, D, 1),
        "dw_saqkv": np.ascontiguousarray(inp["d_sa_qkv_w"].swapaxes(1, 2)).astype(BF16),
        "db_saqkv": inp["d_sa_qkv_b"].astype(f32).reshape(ND, 3 * D, 1),
        "dw_saout": np.ascontiguousarray(inp["d_sa_out_w"].swapaxes(1, 2)).astype(BF16),
        "db_saout": inp["d_sa_out_b"].astype(f32).reshape(ND, D, 1),
        "dw_caqkv": np.ascontiguousarray(inp["d_ca_qkv_w"].swapaxes(1, 2)).astype(BF16),
        "db_caqkv": inp["d_ca_qkv_b"].astype(f32).reshape(ND, 3 * D, 1),
        "dw_caout": np.ascontiguousarray(inp["d_ca_out_w"].swapaxes(1, 2)).astype(BF16),
        "db_caout": inp["d_ca_out_b"].astype(f32).reshape(ND, D, 1),
        "dw_f1": np.ascontiguousarray(inp["d_ff1_w"].swapaxes(1, 2)).astype(BF16),
        "db_f1": inp["d_ff1_b"].astype(f32).reshape(ND, F, 1),
        "dw_f2": np.ascontiguousarray(inp["d_ff2_w"].swapaxes(1, 2)).astype(BF16),
        "db_f2": inp["d_ff2_b"].astype(f32).reshape(ND, D, 1),
        "eb_qkv_bf": inp["e_qkv_b"].astype(BF16).reshape(NE, 3 * D, 1),
        "db_saqkv_bf": inp["d_sa_qkv_b"].astype(BF16).reshape(ND, 3 * D, 1),
        "db_caqkv_bf": inp["d_ca_qkv_b"].astype(BF16).reshape(ND, 3 * D, 1),
        "ident": np.eye(P, dtype=np.float32),
    }

    in_maps = []
    for c in range(NCORE):
        b, half = c // 2, c % 2
        sl = slice(half * TOWN, (half + 1) * TOWN)
        m = dict(W)
        xT = np.ascontiguousarray(inp["enc_in"][b].astype(f32).T[:, sl])
        m["x0T"] = xT.reshape(NDT, P, TOWN)
        yT = np.ascontiguousarray(inp["dec_in"][b].astype(f32).T[:, sl])
        m["y0T"] = yT.reshape(NDT, P, TOWN)
        xyz = inp["enc_xyz"][b].astype(f32)
        n2 = (xyz * xyz).sum(-1, dtype=f32).astype(f32)
        xq2 = (np.float32(2.0) * xyz[sl]).astype(f32)
        m["xq2"] = np.ascontiguousarray(xq2)
        m["xq2row"] = np.ascontiguousarray(xq2.T)
        xkn = np.concatenate([xyz, n2[:, None]], 1).astype(f32)
        m["xkn"] = np.ascontiguousarray(xkn)
        m["xrow"] = np.ascontiguousarray(xkn.T)
        bos = np.full((1, TOWN), 1e30, f32)
        if half == 0:
            bos[0, 0] = NEG
        m["bosrow"] = bos
        qg = np.arange(half * TOWN, (half + 1) * TOWN)
        kg = np.arange(LE)
        m["causal"] = np.ascontiguousarray(
            (kg[:, None] <= qg[None, :]).astype(BF16)
        ).reshape(NKT, P, TOWN)
        in_maps.append(m)
    return in_maps


def assemble(results):
    enc = np.zeros((B, LE, D), np.float32)
    dec = np.zeros((B, LD, D), np.float32)
    for c, r in enumerate(results):
        b, half = c // 2, c % 2
        sl = slice(half * TOWN, (half + 1) * TOWN)
        enc[b, sl, :] = r["enc_part"].reshape(D, TOWN).T
        dec[b, sl, :] = r["dec_part"].reshape(D, TOWN).T
    return enc, dec


def kernel(**inputs):
    from concourse import bass_utils

    if "nc" not in _CACHE:
        _CACHE["nc"] = build()
    nc = _CACHE["nc"]
    in_maps = make_in_maps(inputs)
    enc = dec = None
    for attempt in range(3):
        try:
            res = bass_utils.run_bass_kernel_spmd(
                nc, in_maps, core_ids=list(range(NCORE))
            )
        except Exception:
            if attempt == 2:
                raise
            continue
        enc, dec = assemble(res.results)
        # transient first-execution flakes have shown up as NaN output;
        # the math can never produce NaN, so retry on detection
        if not (np.isnan(enc).any() or np.isnan(dec).any()):
            break
    return enc, dec



# revision 49
# speedup vs baseline: 1.0343x; 1.0204x over previous
"""Trainium2 Bass kernel for nn_EncoderDecoderTransformer (sparse kNN encoder attention).

Sharding: data-parallel over batch (4 batches x 2 cores) with each pair of cores
splitting the sequence dimension (512 tokens each). Per layer, K/V are exchanged
within the pair via AllGather over replica groups [[0,1],[2,3],[4,5],[6,7]].

Layouts (per core):
  - Activations feature-major: x^T stored as 4 tiles (128 dims, 512 own tokens).
  - Q^T/K^T feature-major (head h lives in rows [64*(h%2):...] of ptile h//2).
  - V token-major (128 tokens, 8 heads, 65) with a constant-1 column per head so
    the AV matmul also produces the softmax denominator in psum row 64.
  - Scores computed transposed: S^T = K^T.T @ Q^T  (keys on partitions), exp on
    the Scalar engine, kNN/causal masking as a 0/1 bf16 multiply.
  - kNN mask: s'_qk = 2 x_q.x_k - |x_k|^2 orders like -distance. The 17th
    largest per row (self is always rank 1) is the inclusion threshold; computed
    with the DVE max8/match_replace top-k primitives. Verified to reproduce the
    reference _knn_mask exactly.
"""

import os
import numpy as np
import ml_dtypes

BF16 = ml_dtypes.bfloat16

D, F, H, NE, ND, KNN = 512, 2048, 8, 4, 4, 16
B, LE, LD = 4, 1024, 1024
DH = D // H
NCORE = 8
P = 128
TOWN = 512          # tokens owned per core
NDT = D // P        # 4 feature tiles
NKT = LE // P       # 8 key tiles
NEG = -1e30
EPS = 1e-5
PAIRS = [[0, 1], [2, 3], [4, 5], [6, 7]]

_CACHE = {}


def build(n_enc=NE, n_dec=ND):
    from contextlib import ExitStack

    import concourse.bacc as bacc
    import concourse.tile as tile
    import concourse.mybir as mybir

    f32 = mybir.dt.float32
    bf16 = mybir.dt.bfloat16
    f16 = mybir.dt.float16
    AF = mybir.ActivationFunctionType
    OP = mybir.AluOpType

    import concourse.bass as cbass

    nc = bacc.Bacc("TRN2", target_bir_lowering=False, debug=False, num_devices=NCORE)
    NOGP = not os.environ.get("KQ_GP")  # gpsimd elementwise corrupts data on HW; keep off

    # ---- I/O ----
    def din(name, shape, dt=f32):
        return nc.dram_tensor(name, shape, dt, kind="ExternalInput")

    x0T = din("x0T", [NDT, P, TOWN])
    y0T = din("y0T", [NDT, P, TOWN])
    xq2_d = din("xq2", [TOWN, 3])       # 2*xyz for own tokens
    xq2row_d = din("xq2row", [3, TOWN])  # same, transposed
    xkn_d = din("xkn", [LE, 4])          # [xyz, |xyz|^2] all tokens
    xrow_d = din("xrow", [4, LE])        # same, transposed
    bosrow = din("bosrow", [1, TOWN])
    ident_in = din("ident", [P, P])
    causal_in = din("causal", [NKT, P, TOWN], bf16)

    ew_qkv = din("ew_qkv", [NE, D, 3 * D], bf16)
    ew_out = din("ew_out", [NE, D, D], bf16)
    ew_f1 = din("ew_f1", [NE, D, F], bf16)
    ew_f2 = din("ew_f2", [NE, F, D], bf16)
    eb_qkv = din("eb_qkv", [NE, 3 * D, 1])
    eb_out = din("eb_out", [NE, D, 1])
    eb_f1 = din("eb_f1", [NE, F, 1])
    eb_f2 = din("eb_f2", [NE, D, 1])

    dw_saqkv = din("dw_saqkv", [ND, D, 3 * D], bf16)
    db_saqkv = din("db_saqkv", [ND, 3 * D, 1])
    dw_saout = din("dw_saout", [ND, D, D], bf16)
    db_saout = din("db_saout", [ND, D, 1])
    dw_caqkv = din("dw_caqkv", [ND, D, 3 * D], bf16)
    db_caqkv = din("db_caqkv", [ND, 3 * D, 1])
    dw_caout = din("dw_caout", [ND, D, D], bf16)
    db_caout = din("db_caout", [ND, D, 1])
    dw_f1 = din("dw_f1", [ND, D, F], bf16)
    db_f1 = din("db_f1", [ND, F, 1])
    dw_f2 = din("dw_f2", [ND, F, D], bf16)
    db_f2 = din("db_f2", [ND, D, 1])
    eb_qkv_bf = din("eb_qkv_bf", [NE, 3 * D, 1], bf16)
    db_saqkv_bf = din("db_saqkv_bf", [ND, 3 * D, 1], bf16)
    db_caqkv_bf = din("db_caqkv_bf", [ND, 3 * D, 1], bf16)

    enc_part = nc.dram_tensor("enc_part", [NDT, P, TOWN], f32, kind="ExternalOutput")
    dec_part = nc.dram_tensor("dec_part", [NDT, P, TOWN], f32, kind="ExternalOutput")
    n_dbg = int(os.environ.get("KQ_DEBUG", "0"))
    dbg_t = None
    if n_dbg:
        dbg_t = nc.dram_tensor("dbg", [n_dbg, NDT, P, TOWN], f32, kind="ExternalOutput")
    dbg_i = [0]
    dbgkv_t = None
    if os.environ.get("KQ_DEBUG_KV"):
        dbgkv_t = nc.dram_tensor("dbgkv", [2, 2, 4, P, TOWN], bf16, kind="ExternalOutput")
        dbgk_t = nc.dram_tensor("dbgk", [4, P, 2, TOWN], bf16, kind="ExternalOutput")
        dbgko_t = nc.dram_tensor("dbgko", [4, P, TOWN], bf16, kind="ExternalOutput")
    dbga_t = None
    if os.environ.get("KQ_DEBUG_ALLOW"):
        dbga_t = nc.dram_tensor("dbga", [NKT, P, TOWN], bf16, kind="ExternalOutput")
        dbgs_t = nc.dram_tensor("dbgs", [NKT, P, TOWN], f32, kind="ExternalOutput")
        dbgt_t = nc.dram_tensor("dbgt", [1, TOWN], f32, kind="ExternalOutput")

    with tile.TileContext(nc) as tc, ExitStack() as ctx:
        ep = ctx.enter_context

        pc = ep(tc.tile_pool(name="pc", bufs=1))
        p_allow = ep(tc.tile_pool(name="p_allow", bufs=8))
        p_causal = ep(tc.tile_pool(name="p_causal", bufs=8))
        # PSUM budget (8 banks): ps_s pair-tiles [P,1024] x2 = 4, ps_o [65,512] x2 = 2,
        # ps_mm [*,512] x2 = 2.
        ps_s = ep(tc.tile_pool(name="ps_s", bufs=2, space="PSUM"))
        ps_o = ep(tc.tile_pool(name="ps_o", bufs=2, space="PSUM"))
        ps_mm = ep(tc.tile_pool(name="ps_mm", bufs=2, space="PSUM"))
        p_dram = ep(tc.tile_pool(name="p_dram", bufs=2, space="DRAM"))

        # ---- constants ----
        ones_col = pc.tile([P, 1], f32)          # LN partition sums (lhsT)
        nc.vector.memset(ones_col, 1.0)
        ones_row = pc.tile([1, P], f32)          # broadcast lhsT (K=1)
        nc.vector.memset(ones_row, 1.0)
        ones_row512 = pc.tile([1, TOWN], f32)    # rhs for row-rank1 bias matmuls
        nc.vector.memset(ones_row512, 1.0)
        ones_col_bf = pc.tile([P, 1], bf16)
        nc.vector.memset(ones_col_bf, 1.0)
        ones_row_bf = pc.tile([1, P], bf16)
        nc.vector.memset(ones_row_bf, 1.0)
        ones_row512_bf = pc.tile([1, TOWN], bf16)
        nc.vector.memset(ones_row512_bf, 1.0)
        ones_row_f16 = pc.tile([1, P], f16)
        nc.vector.memset(ones_row_f16, 1.0)
        dummy_w = pc.tile([P, P], bf16)
        nc.vector.memset(dummy_w, 0.0)
        dummy_x = pc.tile([P, TOWN], bf16)
        nc.vector.memset(dummy_x, 0.0)
        eps_sb = pc.tile([1, 1], f32)
        nc.vector.memset(eps_sb, EPS)

        bos_sb = pc.tile([1, TOWN], f32)
        nc.sync.dma_start(out=bos_sb, in_=bosrow[:, :])
        ident_sb = pc.tile([P, P], f32)
        nc.sync.dma_start(out=ident_sb, in_=ident_in[:, :])

        causal_sb = []
        for kt in range(NKT):
            t = p_causal.tile([P, TOWN], bf16, tag="causal")
            nc.sync.dma_start(out=t, in_=causal_in[kt])
            causal_sb.append(t)

        def dbg(ts):
            if dbg_t is None or dbg_i[0] >= n_dbg:
                return
            for dt in range(NDT):
                nc.sync.dma_start(out=dbg_t[dbg_i[0], dt], in_=ts[dt])
            dbg_i[0] += 1

        def pe_warm(n):
            # filler matmuls emitted into PE-idle windows (AG waits, mask
            # phase) so the HAM clock gate stays at full rate; never read
            psD = ps_mm.tile([P, TOWN], f32, tag="mm")
            for _ in range(n):
                nc.tensor.matmul(psD, dummy_w, dummy_x, start=True, stop=True)

        def build_mask():

            # s'_qk = 2 x_q . x_k - |x_k|^2 computed with IEEE-exact fp32 DVE ops
            # (the PE fp32 matmul is not exact fp32 and flips kNN boundary choices).
            # Both layouts use the same per-element op chain => bit-identical values.
            def bcast_rows(dram_row_ap, pool, n_free, tag):
                # (n_free,) DRAM row -> (P, n_free) SBUF tile, replicated across partitions
                t = pool.tile([P, n_free], f32, tag=tag)
                src_ap = cbass.AP(
                    tensor=dram_row_ap.tensor, offset=dram_row_ap.offset,
                    ap=[[0, P]] + list(dram_row_ap.ap),
                )
                nc.sync.dma_start(out=t, in_=src_ap)
                return t

            allow_sb = []
            with tc.tile_pool(name="p_mask", bufs=2) as p_mask, \
                 tc.tile_pool(name="p_mbc", bufs=1) as p_mbc, \
                 tc.tile_pool(name="p_m8", bufs=8) as p_m8:
                bcx = []
                for c in range(4):
                    t = bcast_rows(xrow_d[c], p_mbc, LE, tag=f"bcx{c}")
                    bcx.append(t)
                psT = ps_mm.tile([1, TOWN], f32, tag="mm")
                for qt in range(4):
                    xqc = p_m8.tile([P, 3], f32, tag="xqc")
                    nc.sync.dma_start(out=xqc, in_=xq2_d[qt * P:(qt + 1) * P, :])
                    s0 = p_mask.tile([P, LE], f32, tag="s")
                    nc.vector.tensor_scalar(s0, bcx[0], xqc[:, 0:1], None, op0=OP.mult)
                    s1 = p_mask.tile([P, LE], f32, tag="s")
                    nc.vector.scalar_tensor_tensor(s1, bcx[1], xqc[:, 1:2], s0, OP.mult, OP.add)
                    s2 = p_mask.tile([P, LE], f32, tag="s")
                    nc.vector.scalar_tensor_tensor(s2, bcx[2], xqc[:, 2:3], s1, OP.mult, OP.add)
                    s3 = p_mask.tile([P, LE], f32, tag="s")
                    nc.vector.tensor_tensor(s3, s2, bcx[3], OP.subtract)
                    m8 = p_m8.tile([P, 8], f32, tag="m8")
                    nc.vector.max(m8, s3)
                    s4 = p_mask.tile([P, LE], f32, tag="s")
                    nc.vector.match_replace(s4, m8, s3, NEG)
                    m8b = p_m8.tile([P, 8], f32, tag="m8")
                    nc.vector.max(m8b, s4)
                    s5 = p_mask.tile([P, LE], f32, tag="s")
                    nc.vector.match_replace(s5, m8b, s4, NEG)
                    m8c = p_m8.tile([P, 8], f32, tag="m8")
                    nc.vector.max(m8c, s5)
                    # rank-17 value (16 NN + self) is the inclusion threshold;
                    # PE-transpose the per-query column into one PSUM row
                    # (replaces a 512-descriptor DRAM gather that stalled ~50us)
                    nc.tensor.matmul(
                        psT[0:1, qt * P:(qt + 1) * P], m8c[:, 0:1], ident_sb,
                        is_transpose=True, start=qt == 0, stop=qt == 3,
                        skip_group_check=True,
                    )
                t_row = pc.tile([1, TOWN], f32)
                nc.vector.tensor_copy(t_row, psT)
                t2 = pc.tile([1, TOWN], f32)
                nc.vector.tensor_tensor(t2, t_row, bos_sb, OP.min)
                psBC = ps_mm.tile([P, TOWN], f32, tag="mm")
                nc.tensor.matmul(psBC, ones_row, t2, start=True, stop=True)
                t_bc = pc.tile([P, TOWN], f32)
                nc.vector.tensor_copy(t_bc, psBC)
                bq = []
                for c in range(3):
                    t = bcast_rows(xq2row_d[c], p_mbc, TOWN, tag=f"bq{c}")
                    bq.append(t)
                for kt in range(NKT):
                    xkc = p_m8.tile([P, 4], f32, tag="xkc")
                    nc.sync.dma_start(out=xkc, in_=xkn_d[kt * P:(kt + 1) * P, :])
                    u0 = p_mask.tile([P, TOWN], f32, tag="st")
                    nc.vector.tensor_scalar(u0, bq[0], xkc[:, 0:1], None, op0=OP.mult)
                    u1 = p_mask.tile([P, TOWN], f32, tag="st")
                    nc.vector.scalar_tensor_tensor(u1, bq[1], xkc[:, 1:2], u0, OP.mult, OP.add)
                    u2 = p_mask.tile([P, TOWN], f32, tag="st")
                    nc.vector.scalar_tensor_tensor(u2, bq[2], xkc[:, 2:3], u1, OP.mult, OP.add)
                    u3 = p_mask.tile([P, TOWN], f32, tag="st")
                    nc.vector.tensor_scalar(u3, u2, xkc[:, 3:4], None, op0=OP.subtract)
                    al = p_allow.tile([P, TOWN], bf16, tag="allow")
                    nc.vector.tensor_tensor(al, u3, t_bc, OP.is_ge)
                    if kt == 0:
                        # BOS key allowed for all q; emitted here (not after the
                        # loop) so attention on kt=0 can start immediately
                        nc.vector.memset(al[0:1, :], 1.0)
                    if dbga_t is not None:
                        nc.sync.dma_start(out=dbgs_t[kt], in_=u3)
                    allow_sb.append(al)
                if dbga_t is not None:
                    nc.sync.dma_start(out=dbgt_t[:, :], in_=t2)
                    for kt in range(NKT):
                        nc.sync.dma_start(out=dbga_t[kt], in_=allow_sb[kt])
            return allow_sb



        # ================= helpers =================
        def load_w(pool, dram_ap, kchunks, cols, tag):
            t = pool.tile([P, kchunks, cols], bf16, tag=tag)
            nc.sync.dma_start(
                out=t, in_=dram_ap.rearrange("(kc p) m -> p kc m", p=P)
            )
            return t

        def xshadow(x):
            # bf16 shadow + square, emitted at x-production time so the next
            # layer_norm starts directly with its reduction matmuls
            xb = p_lnsq.tile([P, TOWN], bf16, tag="lnxb")
            nc.vector.tensor_copy(xb, x)
            sq = p_lnsq.tile([P, TOWN], bf16, tag="lnsq")
            nc.scalar.activation(sq, x, AF.Square)
            return xb, sq

        def layer_norm(xst, out_dt, out_pool, out_tag):
            xs, xbs, sqs = xst
            ps_mean = ps_mm.tile([1, TOWN], f32, tag="mm")
            for dt in range(NDT):
                nc.tensor.matmul(ps_mean, ones_col_bf, xbs[dt], start=dt == 0, stop=dt == 3)
            ps_sq = ps_mm.tile([1, TOWN], f32, tag="mm")
            for dt in range(NDT):
                nc.tensor.matmul(ps_sq, ones_col_bf, sqs[dt], start=dt == 0, stop=dt == 3)
            mu16 = p_small.tile([1, TOWN], f16, tag="smf16")
            nc.scalar.mul(mu16, ps_mean, 1.0 / D)
            musq = p_small.tile([1, TOWN], f32, tag="sm")
            nc.scalar.activation(musq, ps_mean, AF.Square, scale=1.0 / D)
            var = p_small.tile([1, TOWN], f32, tag="sm")
            nc.vector.scalar_tensor_tensor(var, ps_sq, 1.0 / D, musq, OP.mult, OP.subtract)
            lnv = p_small.tile([1, TOWN], f32, tag="sm")
            nc.scalar.activation(lnv, var, AF.Ln, bias=eps_sb)
            rstd = p_small.tile([1, TOWN], f16, tag="smf16")
            nc.scalar.activation(rstd, lnv, AF.Exp, scale=-0.5)
            # h = (x - mu)*rstd via f16 rank-1 broadcasts of mu and rstd
            ps_a = ps_mm.tile([P, TOWN], f32, tag="mm")
            nc.tensor.matmul(ps_a, ones_row_f16, rstd, start=True, stop=True)
            ps_c = ps_mm.tile([P, TOWN], f32, tag="mm")
            nc.tensor.matmul(ps_c, ones_row_f16, mu16, start=True, stop=True)
            a_sb = p_lnac.tile([P, TOWN], f32, tag="lna")
            nc.vector.tensor_copy(a_sb, ps_a)
            c_sb = p_lnac.tile([P, TOWN], f32, tag="lnc")
            nc.vector.tensor_copy(c_sb, ps_c)
            hs = []
            for dt in range(NDT):
                h = out_pool.tile([P, TOWN], out_dt, tag=out_tag)
                nc.vector.tensor_tensor(h, xs[dt], c_sb, OP.subtract)
                (nc.vector if NOGP else nc.gpsimd).tensor_tensor(h, h, a_sb, OP.mult)
                hs.append(h)
            return hs

        def proj_fm(w_sb, col_off, n_m, rhs, bias_ap, out_pool, out_tag, out_dt=bf16):
            """Feature-major projection; per-partition bias applied on eviction."""
            outs = []
            nk = len(rhs)
            for m in range(n_m):
                ps = ps_mm.tile([P, TOWN], f32, tag="mm")
                for kc in range(nk):
                    nc.tensor.matmul(
                        ps, w_sb[:, kc, col_off + m * P:col_off + (m + 1) * P],
                        rhs[kc], start=kc == 0, stop=kc == nk - 1,
                    )
                bcol = p_bias.tile([P, 1], f32, tag="bcol")
                nc.sync.dma_start(out=bcol, in_=bias_ap[col_off + m * P:col_off + (m + 1) * P, :])
                o = out_pool.tile([P, TOWN], out_dt, tag=out_tag)
                nc.vector.tensor_scalar(o, ps, bcol, None, op0=OP.add)
                outs.append(o)
            return outs

        def proj_rank1bias(w_sb, col_off, rhs, bias_ap, bias_off, token_major):
            """K^T (feature-major) or V (token-major) projection with the bias
            folded in as a rank-1 bf16 matmul; evicted compact bf16 for the AG bounce."""
            outs = []
            brow512 = None
            if token_major:
                brow512 = p_bias.tile([1, TOWN], bf16, tag="brow512")
                nc.sync.dma_start(
                    out=brow512,
                    in_=bias_ap[bias_off:bias_off + D, :].rearrange("a b -> b a"),
                )
            for m in range(4):
                ps = ps_mm.tile([P, TOWN], f32, tag="mm")
                if token_major:
                    for kc in range(4):
                        nc.tensor.matmul(
                            ps, rhs[kc][:, m * P:(m + 1) * P],
                            w_sb[:, kc, col_off:col_off + D],
                            start=kc == 0, stop=False,
                        )
                    nc.tensor.matmul(ps, ones_row_bf, brow512, start=False, stop=True)
                else:
                    for kc in range(4):
                        nc.tensor.matmul(
                            ps, w_sb[:, kc, col_off + m * P:col_off + (m + 1) * P],
                            rhs[kc], start=kc == 0, stop=False,
                        )
                    brow = p_bias.tile([1, P], bf16, tag="brow")
                    nc.sync.dma_start(
                        out=brow,
                        in_=bias_ap[bias_off + m * P:bias_off + (m + 1) * P, :].rearrange("a b -> b a"),
                    )
                    nc.tensor.matmul(ps, brow, ones_row512_bf, start=False, stop=True)
                o = p_kvout.tile([P, TOWN], bf16, tag="kvout")
                nc.vector.tensor_copy(o, ps)
                outs.append(o)
            return outs

        def kv_allgather(k_outs, v_outs, uniq):
            # one DRAM buffer set per layer: a pool-recycled buffer could be
            # rewritten while the pair-peer's collective still reads it
            # (cross-core hazard invisible to Tile's per-core dep tracking).
            # K and V go in separate AllGathers so attention (which needs K
            # first) can start while V is still in flight.
            bin_k = p_dram.tile([4, P, TOWN], bf16, tag=f"agink{uniq}")
            for p in range(4):
                nc.sync.dma_start(out=bin_k[p], in_=k_outs[p])
            bout_k = p_dram.tile([2, 4, P, TOWN], bf16, tag=f"agoutk{uniq}")
            nc.gpsimd.collective_compute(
                "AllGather", OP.bypass, replica_groups=PAIRS,
                ins=[bin_k[:].opt()], outs=[bout_k[:].opt()],
            )
            bin_v = p_dram.tile([4, P, TOWN], bf16, tag=f"aginv{uniq}")
            for p in range(4):
                nc.sync.dma_start(out=bin_v[p], in_=v_outs[p])
            bout_v = p_dram.tile([2, 4, P, TOWN], bf16, tag=f"agoutv{uniq}")
            nc.gpsimd.collective_compute(
                "AllGather", OP.bypass, replica_groups=PAIRS,
                ins=[bin_v[:].opt()], outs=[bout_v[:].opt()],
            )
            return (bout_k, bout_v)

        def load_k(bout):
            Ks = []
            for p in range(4):
                kt = p_kv.tile([P, 2, TOWN], bf16, tag="ksb")
                for r in range(2):
                    nc.sync.dma_start(out=kt[:, r, :], in_=bout[r, p])
                Ks.append(kt)
            return Ks

        def load_v(bout):
            Vs = []
            for r in range(2):
                for tt in range(4):
                    vt = p_v.tile([P, H, 65], bf16, tag="vsb")
                    nc.sync.dma_start(
                        out=vt[:, :, 0:64],
                        in_=bout[r, tt].rearrange("p (h d) -> p h d", h=H),
                    )
                    (nc.vector if NOGP else nc.gpsimd).memset(vt[:, :, 64:65], 1.0)
                    Vs.append(vt)
            return Vs

        def attention(Qs, Ks, Vs, mask_tiles):
            # Scores for the head pair land in one 2-bank PSUM tile [P, 2*TOWN]
            # (j=0 cols 0:TOWN, j=1 cols TOWN:2*TOWN) so exp and the mask
            # multiply run once per (hp, kt) at N=1024 instead of twice at
            # N=512 — the serial exp->mask chain was the attention rate
            # limiter. psO is evicted to SBUF with a single copy so the two
            # PSUM banks recycle quickly.
            OTs = []
            for hp in range(4):
                psO = []
                for _j in range(2):
                    psO_t = ps_o.tile([65, TOWN], f32, tag="pso")
                    psO.append(psO_t)
                for kt in range(NKT):
                    r, c = kt // 4, kt % 4
                    psP = ps_s.tile([P, 2 * TOWN], f32, tag="pss")
                    for j in range(2):
                        rows = slice(j * 64, (j + 1) * 64)
                        nc.tensor.matmul(
                            psP[:, j * TOWN:(j + 1) * TOWN],
                            Ks[hp][rows, r, c * P:(c + 1) * P], Qs[hp][rows, :],
                            start=True, stop=True,
                        )
                    e = p_e.tile([P, 2 * TOWN], bf16, tag="e")
                    nc.scalar.activation(e, psP, AF.Exp, scale=0.125)
                    if mask_tiles is not None:
                        # one multiply over both halves; the [P, TOWN] mask is
                        # repeated via a stride-0 middle AP dim
                        m_ap = mask_tiles[kt][:, :]
                        m_rep = cbass.AP(
                            tensor=m_ap.tensor, offset=m_ap.offset,
                            ap=[list(m_ap.ap[0]), [0, 2]] + [list(d) for d in m_ap.ap[1:]],
                        )
                        e_ap = e[:, :]
                        e_v = cbass.AP(
                            tensor=e_ap.tensor, offset=e_ap.offset,
                            ap=[list(e_ap.ap[0]), [TOWN, 2], [1, TOWN]],
                        )
                        nc.vector.tensor_tensor(e_v, e_v, m_rep, OP.mult)
                    for j in range(2):
                        head = 2 * hp + j
                        nc.tensor.matmul(
                            psO[j], Vs[kt][:, head, :], e[:, j * TOWN:(j + 1) * TOWN],
                            start=kt == 0, stop=kt == NKT - 1,
                        )
                ot = p_ot.tile([P, TOWN], bf16, tag="ot")
                for j in range(2):
                    oraw = p_oraw.tile([65, TOWN], f32, tag="oraw")
                    nc.vector.tensor_copy(oraw, psO[j])
                    den = p_small.tile([1, TOWN], f32, tag="sm")
                    nc.vector.tensor_copy(den, oraw[64:65, :])
                    rec = p_small.tile([1, TOWN], f32, tag="sm")
                    nc.vector.reciprocal_approx_fast(rec, den)
                    # broadcast rec across 64 partitions via a DRAM round-trip
                    # (frees PE + DVE of the rank-1 matmul + psum eviction; the
                    # latency hides under the next head-pair's kt loop)
                    recd = p_dram.tile([1, TOWN], f32, tag="recd")
                    nc.sync.dma_start(out=recd, in_=rec)
                    bc = p_bc.tile([64, TOWN], f32, tag="bc")
                    rap = recd[0]
                    nc.sync.dma_start(out=bc, in_=cbass.AP(
                        tensor=rap.tensor, offset=rap.offset,
                        ap=[[0, 64]] + [list(d) for d in rap.ap],
                    ))
                    nc.vector.tensor_tensor(ot[j * 64:(j + 1) * 64, :], oraw[0:64, :], bc, OP.mult)
                OTs.append(ot)
            return OTs

        def proj_residual(w_sb, col_off, n_k, rhs, bias_ap, xs):
            nxs, nxbs, nsqs = [], [], []
            for m in range(NDT):
                ps = ps_mm.tile([P, TOWN], f32, tag="mm")
                for kc in range(n_k):
                    nc.tensor.matmul(
                        ps, w_sb[:, kc, col_off + m * P:col_off + (m + 1) * P],
                        rhs[kc], start=kc == 0, stop=kc == n_k - 1,
                    )
                bcol = p_bias.tile([P, 1], f32, tag="bcol")
                nc.sync.dma_start(out=bcol, in_=bias_ap[m * P:(m + 1) * P, :])
                nx = p_x.tile([P, TOWN], f32, tag="x")
                nc.vector.scalar_tensor_tensor(nx, ps, bcol, xs[m], OP.add, OP.add)
                nxs.append(nx)
                xb, sq = xshadow(nx)
                nxbs.append(xb); nsqs.append(sq)
            return nxs, nxbs, nsqs

        def ffn(w1_ap, w2_ap, b1_ap, b2_ap, hs, xs):
            nxbs, nsqs = [], []
            gs = []
            for m in range(F // P):
                w1m = p_w1.tile([P, NDT, P], bf16, tag="wf1")
                nc.sync.dma_start(
                    out=w1m,
                    in_=w1_ap[:, m * P:(m + 1) * P].rearrange("(kc p) m -> p kc m", p=P),
                )
                ps = ps_mm.tile([P, TOWN], f32, tag="mm")
                for kc in range(NDT):
                    nc.tensor.matmul(
                        ps, w1m[:, kc, :], hs[kc],
                        start=kc == 0, stop=kc == NDT - 1,
                    )
                bcol = p_bias.tile([P, 1], f32, tag="bcol")
                nc.sync.dma_start(out=bcol, in_=b1_ap[m * P:(m + 1) * P, :])
                g = p_g.tile([P, TOWN], bf16, tag="g")
                nc.scalar.activation(g, ps, AF.Gelu, bias=bcol)
                gs.append(g)
            nxs = []
            for m in range(NDT):
                w2m = p_w2.tile([P, F // P, P], bf16, tag="wf2")
                nc.sync.dma_start(
                    out=w2m,
                    in_=w2_ap[:, m * P:(m + 1) * P].rearrange("(kc p) c -> p kc c", p=P),
                )
                ps2 = ps_mm.tile([P, TOWN], f32, tag="mm")
                for kc in range(F // P):
                    nc.tensor.matmul(
                        ps2, w2m[:, kc, :], gs[kc],
                        start=kc == 0, stop=kc == F // P - 1,
                    )
                bcol = p_bias.tile([P, 1], f32, tag="bcol")
                nc.sync.dma_start(out=bcol, in_=b2_ap[m * P:(m + 1) * P, :])
                nx = p_x.tile([P, TOWN], f32, tag="x")
                nc.vector.scalar_tensor_tensor(nx, ps2, bcol, xs[m], OP.add, OP.add)
                nxs.append(nx)
                xb, sq = xshadow(nx)
                nxbs.append(xb); nsqs.append(sq)
            return nxs, nxbs, nsqs

        p_x = ep(tc.tile_pool(name="p_x", bufs=5))
        p_h = ep(tc.tile_pool(name="p_h", bufs=6))
        p_q = ep(tc.tile_pool(name="p_q", bufs=5))
        p_kv = ep(tc.tile_pool(name="p_kv", bufs=4))
        p_v = ep(tc.tile_pool(name="p_v", bufs=8))
        p_kvout = ep(tc.tile_pool(name="p_kvout", bufs=4))
        p_ot = ep(tc.tile_pool(name="p_ot", bufs=4))
        p_e = ep(tc.tile_pool(name="p_e", bufs=2))
        p_oraw = ep(tc.tile_pool(name="p_oraw", bufs=2))
        p_g = ep(tc.tile_pool(name="p_g", bufs=16))
        p_lnsq = ep(tc.tile_pool(name="p_lnsq", bufs=4))
        p_lnac = ep(tc.tile_pool(name="p_lnac", bufs=1))
        p_bc = ep(tc.tile_pool(name="p_bc", bufs=2))
        p_small = ep(tc.tile_pool(name="p_small", bufs=3))
        p_bias = ep(tc.tile_pool(name="p_bias", bufs=4))
        p_eo = ep(tc.tile_pool(name="p_eo", bufs=4))
        p_eob = ep(tc.tile_pool(name="p_eob", bufs=4))
        p_w1 = ep(tc.tile_pool(name="p_w1", bufs=3))
        p_w2 = ep(tc.tile_pool(name="p_w2", bufs=2))
        p_wqkv = ep(tc.tile_pool(name="p_wqkv", bufs=1))
        p_wout = ep(tc.tile_pool(name="p_wout", bufs=2))

        # ================= encoder =================
        xs = []
        xbs, sqs = [], []
        for dt in range(NDT):
            x = p_x.tile([P, TOWN], f32, tag="x")
            nc.sync.dma_start(out=x, in_=x0T[dt])
            xs.append(x)
            xb, sq = xshadow(x)
            xbs.append(xb); sqs.append(sq)
        xst = (xs, xbs, sqs)

        for l in range(n_enc):
            wqkv = load_w(p_wqkv, ew_qkv[l], NDT, 3 * D, "wqkv")
            wout = load_w(p_wout, ew_out[l], NDT, D, "wout")

            hs = layer_norm(xst, bf16, p_h, "h")
            kouts = proj_rank1bias(wqkv, D, hs, eb_qkv_bf[l], D, token_major=False)
            vouts = proj_rank1bias(wqkv, 2 * D, hs, eb_qkv_bf[l], 2 * D, token_major=True)
            boutk, boutv = kv_allgather(kouts, vouts, f"e{l}")
            Qs = proj_fm(wqkv, 0, 4, hs, eb_qkv[l], p_q, "q")
            if l == 0:
                # decoder layer-0 self-attention K/V depends only on dec_in:
                # project + fire its AllGather here (after e0's AG so the
                # collective queue serves e0 first), then build the kNN mask.
                # All of it overlaps the e0 AllGather wait + mask window.
                yt0 = []
                ybt0, sqt0 = [], []
                for dt in range(NDT):
                    y = p_eo.tile([P, TOWN], f32, tag="eof")
                    nc.sync.dma_start(out=y, in_=y0T[dt])
                    yt0.append(y)
                    yb_, sq_ = xshadow(y)
                    ybt0.append(yb_); sqt0.append(sq_)
                hs_d0 = layer_norm((yt0, ybt0, sqt0), bf16, p_eob, "eob")
                wq_d0 = load_w(p_wqkv, dw_saqkv[0], NDT, 3 * D, "wqkv")
                k_d0 = proj_rank1bias(wq_d0, D, hs_d0, db_saqkv_bf[0], D, token_major=False)
                v_d0 = proj_rank1bias(wq_d0, 2 * D, hs_d0, db_saqkv_bf[0], 2 * D, token_major=True)
                d0_bouts = kv_allgather(k_d0, v_d0, "d0")
                hd0_dram = p_dram.tile([NDT, P, TOWN], bf16, tag="hd0")
                for dt in range(NDT):
                    nc.sync.dma_start(out=hd0_dram[dt], in_=hs_d0[dt])
                allow_sb = build_mask()
            pe_warm(60)
            Ks = load_k(boutk)
            Vs = load_v(boutv)
            if dbgkv_t is not None and l == 1:
                for p in range(4):
                    nc.sync.dma_start(out=dbgk_t[p], in_=Ks[p])
                    nc.sync.dma_start(out=dbgko_t[p], in_=kouts[p])
            OTs = attention(Qs, Ks, Vs, allow_sb)
            xst = proj_residual(wout, 0, NDT, OTs, eb_out[l], xst[0])
            dbg(xst[0])
            hs = layer_norm(xst, bf16, p_h, "h")
            xst = ffn(ew_f1[l], ew_f2[l], eb_f1[l], eb_f2[l], hs, xst[0])
            dbg(xst[0])

        pe_warm(12)
        eof = layer_norm(xst, f32, p_eo, "eof")
        eob = []
        for dt in range(NDT):
            nc.sync.dma_start(out=enc_part[dt], in_=eof[dt])
            t = p_eob.tile([P, TOWN], bf16, tag="eob")
            nc.vector.tensor_copy(t, eof[dt])
            eob.append(t)

        # ==== cross-attention K/V: project now, AllGather per decoder layer ====
        bin_cas = []
        with tc.tile_pool(name="p_wca", bufs=1) as p_wca:
            for l in range(n_dec):
                wkv = p_wca.tile([P, NDT, 2 * D], bf16, tag="wcakv")
                nc.sync.dma_start(
                    out=wkv,
                    in_=dw_caqkv[l][:, D:3 * D].rearrange("(kc p) m -> p kc m", p=P),
                )
                kouts = proj_rank1bias(wkv, 0, eob, db_caqkv_bf[l], D, token_major=False)
                vouts = proj_rank1bias(wkv, D, eob, db_caqkv_bf[l], 2 * D, token_major=True)
                bin_k = p_dram.tile([4, P, TOWN], bf16, tag=f"caink{l}")
                bin_v = p_dram.tile([4, P, TOWN], bf16, tag=f"cainv{l}")
                for p in range(4):
                    nc.sync.dma_start(out=bin_k[p], in_=kouts[p])
                    nc.sync.dma_start(out=bin_v[p], in_=vouts[p])
                bin_cas.append((bin_k, bin_v))

        bout_cas = []
        for l in range(n_dec):
            bk = p_dram.tile([2, 4, P, TOWN], bf16, tag=f"caoutk{l}")
            bv = p_dram.tile([2, 4, P, TOWN], bf16, tag=f"caoutv{l}")
            bout_cas.append((bk, bv))
        fire_cross_ag0_early = True

        def fire_cross_ag(l):
            for i in range(2):
                nc.gpsimd.collective_compute(
                    "AllGather", OP.bypass, replica_groups=PAIRS,
                    ins=[bin_cas[l][i][:].opt()], outs=[bout_cas[l][i][:].opt()],
                )

        # ================= decoder =================
        ys = []
        ybs, qsq = [], []
        for dt in range(NDT):
            y = p_x.tile([P, TOWN], f32, tag="x")
            nc.sync.dma_start(out=y, in_=y0T[dt])
            ys.append(y)
            yb, sq_ = xshadow(y)
            ybs.append(yb); qsq.append(sq_)
        yst = (ys, ybs, qsq)

        fire_cross_ag(0)
        for l in range(n_dec):
            wqkv = load_w(p_wqkv, dw_saqkv[l], NDT, 3 * D, "wqkv")
            wout = load_w(p_wout, dw_saout[l], NDT, D, "wout")

            # self-attention (causal)
            if l == 0:
                # K/V AllGather was prefired at program start; reload h
                hs = []
                for dt in range(NDT):
                    h = p_h.tile([P, TOWN], bf16, tag="h")
                    nc.sync.dma_start(out=h, in_=hd0_dram[dt])
                    hs.append(h)
                boutk, boutv = d0_bouts
            else:
                hs = layer_norm(yst, bf16, p_h, "h")
                kouts = proj_rank1bias(wqkv, D, hs, db_saqkv_bf[l], D, token_major=False)
                vouts = proj_rank1bias(wqkv, 2 * D, hs, db_saqkv_bf[l], 2 * D, token_major=True)
                boutk, boutv = kv_allgather(kouts, vouts, f"d{l}")
            Qs = proj_fm(wqkv, 0, 4, hs, db_saqkv[l], p_q, "q")
            pe_warm(60)
            Ks = load_k(boutk)
            Vs = load_v(boutv)
            OTs = attention(Qs, Ks, Vs, causal_sb)
            if l + 1 < n_dec:
                fire_cross_ag(l + 1)
            bout_ca = bout_cas[l]
            yst = proj_residual(wout, 0, NDT, OTs, db_saout[l], yst[0])
            dbg(yst[0])

            # cross-attention (no mask)
            wcaq = load_w(p_wout, dw_caqkv[l][:, 0:D], NDT, D, "wout")
            wcao = load_w(p_wout, dw_caout[l], NDT, D, "wout")
            hs = layer_norm(yst, bf16, p_h, "h")
            Qs = proj_fm(wcaq, 0, 4, hs, db_caqkv[l], p_q, "q")
            Ks = load_k(bout_ca[0])
            Vs = load_v(bout_ca[1])
            OTs = attention(Qs, Ks, Vs, None)
            yst = proj_residual(wcao, 0, NDT, OTs, db_caout[l], yst[0])
            dbg(yst[0])

            # ffn
            hs = layer_norm(yst, bf16, p_h, "h")
            yst = ffn(dw_f1[l], dw_f2[l], db_f1[l], db_f2[l], hs, yst[0])
            dbg(yst[0])

        dof = layer_norm(yst, f32, p_eo, "eof")
        for dt in range(NDT):
            nc.sync.dma_start(out=dec_part[dt], in_=dof[dt])

    nc.compile()
    return nc


def make_in_maps(inputs):
    inp = {k: np.asarray(v) for k, v in inputs.items()}
    f32 = np.float32

    W = {
        "ew_qkv": np.ascontiguousarray(inp["e_qkv_w"].swapaxes(1, 2)).astype(BF16),
        "ew_out": np.ascontiguousarray(inp["e_out_w"].swapaxes(1, 2)).astype(BF16),
        "ew_f1": np.ascontiguousarray(inp["e_ff1_w"].swapaxes(1, 2)).astype(BF16),
        "ew_f2": np.ascontiguousarray(inp["e_ff2_w"].swapaxes(1, 2)).astype(BF16),
        "eb_qkv": inp["e_qkv_b"].astype(f32).reshape(NE, 3 * D, 1),
        "eb_out": inp["e_out_b"].astype(f32).reshape(NE, D, 1),
        "eb_f1": inp["e_ff1_b"].astype(f32).reshape(NE, F, 1),
        "eb_f2": inp["e_ff2_b"].astype(f32).reshape(NE, D, 1),
        "dw_saqkv": np.ascontiguousarray(inp["d_sa_qkv_w"].swapaxes(1, 2)).astype(BF16),
        "db_saqkv": inp["d_sa_qkv_b"].astype(f32).reshape(ND, 3 * D, 1),
        "dw_saout": np.ascontiguousarray(inp["d_sa_out_w"].swapaxes(1, 2)).astype(BF16),
        "db_saout": inp["d_sa_out_b"].astype(f32).reshape(ND, D, 1),
        "dw_caqkv": np.ascontiguousarray(inp["d_ca_qkv_w"].swapaxes(1, 2)).astype(BF16),
        "db_caqkv": inp["d_ca_qkv_b"].astype(f32).reshape(ND, 3 * D, 1),
        "dw_caout": np.ascontiguousarray(inp["d_ca_out_w"].swapaxes(1, 2)).astype(BF16),
        "db_caout": inp["d_ca_out_b"].astype(f32).reshape(ND, D, 1),
        "dw_f1": np.ascontiguousarray(inp["d_ff1_w"].swapaxes(1, 2)).astype(BF16),
        "db_f1": inp["d_ff1_b"].astype(f32).reshape(ND, F, 1),
        "dw_f2": np.ascontiguousarray(inp["d_ff2_w"].swapaxes(1, 2)).astype(BF16),
        "db_f2": inp["d_ff2_b"].astype(f32).reshape(ND, D, 1),
        "eb_qkv_bf": inp["e_qkv_b"].astype(BF16).reshape(NE, 3 * D, 1),
        "db_saqkv_bf": inp["d_sa_qkv_b"].astype(BF16).reshape(ND, 3 * D, 1),
        "db_caqkv_bf": inp["d_ca_qkv_b"].astype(BF16).reshape(ND, 3 * D, 1),
        "ident": np.eye(P, dtype=np.float32),
    }

    in_maps = []
    for c in range(NCORE):
        b, half = c // 2, c % 2
        sl = slice(half * TOWN, (half + 1) * TOWN)
        m = dict(W)
        xT = np.ascontiguousarray(inp["enc_in"][b].astype(f32).T[:, sl])
        m["x0T"] = xT.reshape(NDT, P, TOWN)
        yT = np.ascontiguousarray(inp["dec_in"][b].astype(f32).T[:, sl])
        m["y0T"] = yT.reshape(NDT, P, TOWN)
        xyz = inp["enc_xyz"][b].astype(f32)
        n2 = (xyz * xyz).sum(-1, dtype=f32).astype(f32)
        xq2 = (np.float32(2.0) * xyz[sl]).astype(f32)
        m["xq2"] = np.ascontiguousarray(xq2)
        m["xq2row"] = np.ascontiguousarray(xq2.T)
        xkn = np.concatenate([xyz, n2[:, None]], 1).astype(f32)
        m["xkn"] = np.ascontiguousarray(xkn)
        m["xrow"] = np.ascontiguousarray(xkn.T)
        bos = np.full((1, TOWN), 1e30, f32)
        if half == 0:
            bos[0, 0] = NEG
        m["bosrow"] = bos
        qg = np.arange(half * TOWN, (half + 1) * TOWN)
        kg = np.arange(LE)
        m["causal"] = np.ascontiguousarray(
            (kg[:, None] <= qg[None, :]).astype(BF16)
        ).reshape(NKT, P, TOWN)
        in_maps.append(m)
    return in_maps


def assemble(results):
    enc = np.zeros((B, LE, D), np.float32)
    dec = np.zeros((B, LD, D), np.float32)
    for c, r in enumerate(results):
        b, half = c // 2, c % 2
        sl = slice(half * TOWN, (half + 1) * TOWN)
        enc[b, sl, :] = r["enc_part"].reshape(D, TOWN).T
        dec[b, sl, :] = r["dec_part"].reshape(D, TOWN).T
    return enc, dec


def kernel(**inputs):
    from concourse import bass_utils

    if "nc" not in _CACHE:
        _CACHE["nc"] = build()
    nc = _CACHE["nc"]
    in_maps = make_in_maps(inputs)
    enc = dec = None
    for attempt in range(3):
        try:
            res = bass_utils.run_bass_kernel_spmd(
                nc, in_maps, core_ids=list(range(NCORE))
            )
        except Exception:
            if attempt == 2:
                raise
            continue
        enc, dec = assemble(res.results)
        # transient first-execution flakes have shown up as NaN output;
        # the math can never produce NaN, so retry on detection
        if not (np.isnan(enc).any() or np.isnan(dec).any()):
            break
    return enc, dec



# revision 50
# speedup vs baseline: 1.0614x; 1.0262x over previous
"""Trainium2 Bass kernel for nn_EncoderDecoderTransformer (sparse kNN encoder attention).

Sharding: data-parallel over batch (4 batches x 2 cores) with each pair of cores
splitting the sequence dimension (512 tokens each). Per layer, K/V are exchanged
within the pair via AllGather over replica groups [[0,1],[2,3],[4,5],[6,7]].

Layouts (per core):
  - Activations feature-major: x^T stored as 4 tiles (128 dims, 512 own tokens).
  - Q^T/K^T feature-major (head h lives in rows [64*(h%2):...] of ptile h//2).
  - V token-major (128 tokens, 8 heads, 65) with a constant-1 column per head so
    the AV matmul also produces the softmax denominator in psum row 64.
  - Scores computed transposed: S^T = K^T.T @ Q^T  (keys on partitions), exp on
    the Scalar engine, kNN/causal masking as a 0/1 bf16 multiply.
  - kNN mask: s'_qk = 2 x_q.x_k - |x_k|^2 orders like -distance. The 17th
    largest per row (self is always rank 1) is the inclusion threshold; computed
    with the DVE max8/match_replace top-k primitives. Verified to reproduce the
    reference _knn_mask exactly.
"""

import os
import numpy as np
import ml_dtypes

BF16 = ml_dtypes.bfloat16

D, F, H, NE, ND, KNN = 512, 2048, 8, 4, 4, 16
B, LE, LD = 4, 1024, 1024
DH = D // H
NCORE = 8
P = 128
TOWN = 512          # tokens owned per core
NDT = D // P        # 4 feature tiles
NKT = LE // P       # 8 key tiles
NEG = -1e30
EPS = 1e-5
PAIRS = [[0, 1], [2, 3], [4, 5], [6, 7]]

_CACHE = {}


def build(n_enc=NE, n_dec=ND):
    from contextlib import ExitStack

    import concourse.bacc as bacc
    import concourse.tile as tile
    import concourse.mybir as mybir

    f32 = mybir.dt.float32
    bf16 = mybir.dt.bfloat16
    f16 = mybir.dt.float16
    AF = mybir.ActivationFunctionType
    OP = mybir.AluOpType

    import concourse.bass as cbass

    nc = bacc.Bacc("TRN2", target_bir_lowering=False, debug=False, num_devices=NCORE)
    NOGP = not os.environ.get("KQ_GP")  # gpsimd elementwise corrupts data on HW; keep off

    # ---- I/O ----
    def din(name, shape, dt=f32):
        return nc.dram_tensor(name, shape, dt, kind="ExternalInput")

    x0T = din("x0T", [NDT, P, TOWN])
    y0T = din("y0T", [NDT, P, TOWN])
    xq2_d = din("xq2", [TOWN, 3])       # 2*xyz for own tokens
    xq2row_d = din("xq2row", [3, TOWN])  # same, transposed
    xkn_d = din("xkn", [LE, 4])          # [xyz, |xyz|^2] all tokens
    xrow_d = din("xrow", [4, LE])        # same, transposed
    bosrow = din("bosrow", [1, TOWN])
    ident_in = din("ident", [P, P])
    causal_in = din("causal", [NKT, P, TOWN], bf16)

    ew_qkv = din("ew_qkv", [NE, D, 3 * D], bf16)
    ew_out = din("ew_out", [NE, D, D], bf16)
    ew_f1 = din("ew_f1", [NE, D, F], bf16)
    ew_f2 = din("ew_f2", [NE, F, D], bf16)
    eb_qkv = din("eb_qkv", [NE, 3 * D, 1])
    eb_out = din("eb_out", [NE, D, 1])
    eb_f1 = din("eb_f1", [NE, F, 1])
    eb_f2 = din("eb_f2", [NE, D, 1])

    dw_saqkv = din("dw_saqkv", [ND, D, 3 * D], bf16)
    db_saqkv = din("db_saqkv", [ND, 3 * D, 1])
    dw_saout = din("dw_saout", [ND, D, D], bf16)
    db_saout = din("db_saout", [ND, D, 1])
    dw_caqkv = din("dw_caqkv", [ND, D, 3 * D], bf16)
    db_caqkv = din("db_caqkv", [ND, 3 * D, 1])
    dw_caout = din("dw_caout", [ND, D, D], bf16)
    db_caout = din("db_caout", [ND, D, 1])
    dw_f1 = din("dw_f1", [ND, D, F], bf16)
    db_f1 = din("db_f1", [ND, F, 1])
    dw_f2 = din("dw_f2", [ND, F, D], bf16)
    db_f2 = din("db_f2", [ND, D, 1])
    eb_qkv_bf = din("eb_qkv_bf", [NE, 3 * D, 1], bf16)
    db_saqkv_bf = din("db_saqkv_bf", [ND, 3 * D, 1], bf16)
    db_caqkv_bf = din("db_caqkv_bf", [ND, 3 * D, 1], bf16)

    enc_part = nc.dram_tensor("enc_part", [NDT, P, TOWN], f32, kind="ExternalOutput")
    dec_part = nc.dram_tensor("dec_part", [NDT, P, TOWN], f32, kind="ExternalOutput")
    n_dbg = int(os.environ.get("KQ_DEBUG", "0"))
    dbg_t = None
    if n_dbg:
        dbg_t = nc.dram_tensor("dbg", [n_dbg, NDT, P, TOWN], f32, kind="ExternalOutput")
    dbg_i = [0]
    dbgkv_t = None
    if os.environ.get("KQ_DEBUG_KV"):
        dbgkv_t = nc.dram_tensor("dbgkv", [2, 2, 4, P, TOWN], bf16, kind="ExternalOutput")
        dbgk_t = nc.dram_tensor("dbgk", [4, P, 2, TOWN], bf16, kind="ExternalOutput")
        dbgko_t = nc.dram_tensor("dbgko", [4, P, TOWN], bf16, kind="ExternalOutput")
    dbga_t = None
    if os.environ.get("KQ_DEBUG_ALLOW"):
        dbga_t = nc.dram_tensor("dbga", [NKT, P, TOWN], bf16, kind="ExternalOutput")
        dbgs_t = nc.dram_tensor("dbgs", [NKT, P, TOWN], f32, kind="ExternalOutput")
        dbgt_t = nc.dram_tensor("dbgt", [1, TOWN], f32, kind="ExternalOutput")

    with tile.TileContext(nc) as tc, ExitStack() as ctx:
        ep = ctx.enter_context

        pc = ep(tc.tile_pool(name="pc", bufs=1))
        p_allow = ep(tc.tile_pool(name="p_allow", bufs=8))
        p_causal = ep(tc.tile_pool(name="p_causal", bufs=8))
        # PSUM budget (8 banks): ps_s pair-tiles [P,1024] x2 = 4, ps_o [65,512] x2 = 2,
        # ps_mm [*,512] x2 = 2.
        ps_s = ep(tc.tile_pool(name="ps_s", bufs=2, space="PSUM"))
        ps_o = ep(tc.tile_pool(name="ps_o", bufs=2, space="PSUM"))
        ps_mm = ep(tc.tile_pool(name="ps_mm", bufs=2, space="PSUM"))
        p_dram = ep(tc.tile_pool(name="p_dram", bufs=2, space="DRAM"))

        # ---- constants ----
        ones_col = pc.tile([P, 1], f32)          # LN partition sums (lhsT)
        nc.vector.memset(ones_col, 1.0)
        ones_row = pc.tile([1, P], f32)          # broadcast lhsT (K=1)
        nc.vector.memset(ones_row, 1.0)
        ones_row512 = pc.tile([1, TOWN], f32)    # rhs for row-rank1 bias matmuls
        nc.vector.memset(ones_row512, 1.0)
        ones_col_bf = pc.tile([P, 1], bf16)
        nc.vector.memset(ones_col_bf, 1.0)
        ones_row_bf = pc.tile([1, P], bf16)
        nc.vector.memset(ones_row_bf, 1.0)
        ones_row512_bf = pc.tile([1, TOWN], bf16)
        nc.vector.memset(ones_row512_bf, 1.0)
        ones_row_f16 = pc.tile([1, P], f16)
        nc.vector.memset(ones_row_f16, 1.0)
        dummy_w = pc.tile([P, P], bf16)
        nc.vector.memset(dummy_w, 0.0)
        dummy_x = pc.tile([P, TOWN], bf16)
        nc.vector.memset(dummy_x, 0.0)
        eps_sb = pc.tile([1, 1], f32)
        nc.vector.memset(eps_sb, EPS)

        bos_sb = pc.tile([1, TOWN], f32)
        nc.sync.dma_start(out=bos_sb, in_=bosrow[:, :])
        ident_sb = pc.tile([P, P], f32)
        nc.sync.dma_start(out=ident_sb, in_=ident_in[:, :])

        causal_sb = []
        for kt in range(NKT):
            t = p_causal.tile([P, TOWN], bf16, tag="causal")
            nc.sync.dma_start(out=t, in_=causal_in[kt])
            causal_sb.append(t)

        def dbg(ts):
            if dbg_t is None or dbg_i[0] >= n_dbg:
                return
            for dt in range(NDT):
                nc.sync.dma_start(out=dbg_t[dbg_i[0], dt], in_=ts[dt])
            dbg_i[0] += 1

        def pe_warm(n):
            # filler matmuls emitted into PE-idle windows (AG waits, mask
            # phase) so the HAM clock gate stays at full rate; never read
            psD = ps_mm.tile([P, TOWN], f32, tag="mm")
            for _ in range(n):
                nc.tensor.matmul(psD, dummy_w, dummy_x, start=True, stop=True)

        def build_mask():

            # s'_qk = 2 x_q . x_k - |x_k|^2 computed with IEEE-exact fp32 DVE ops
            # (the PE fp32 matmul is not exact fp32 and flips kNN boundary choices).
            # Both layouts use the same per-element op chain => bit-identical values.
            def bcast_rows(dram_row_ap, pool, n_free, tag):
                # (n_free,) DRAM row -> (P, n_free) SBUF tile, replicated across partitions
                t = pool.tile([P, n_free], f32, tag=tag)
                src_ap = cbass.AP(
                    tensor=dram_row_ap.tensor, offset=dram_row_ap.offset,
                    ap=[[0, P]] + list(dram_row_ap.ap),
                )
                nc.sync.dma_start(out=t, in_=src_ap)
                return t

            allow_sb = []
            with tc.tile_pool(name="p_mask", bufs=2) as p_mask, \
                 tc.tile_pool(name="p_mbc", bufs=1) as p_mbc, \
                 tc.tile_pool(name="p_m8", bufs=8) as p_m8:
                bcx = []
                for c in range(4):
                    t = bcast_rows(xrow_d[c], p_mbc, LE, tag=f"bcx{c}")
                    bcx.append(t)
                psT = ps_mm.tile([1, TOWN], f32, tag="mm")
                for qt in range(4):
                    xqc = p_m8.tile([P, 3], f32, tag="xqc")
                    nc.sync.dma_start(out=xqc, in_=xq2_d[qt * P:(qt + 1) * P, :])
                    s0 = p_mask.tile([P, LE], f32, tag="s")
                    nc.vector.tensor_scalar(s0, bcx[0], xqc[:, 0:1], None, op0=OP.mult)
                    s1 = p_mask.tile([P, LE], f32, tag="s")
                    nc.vector.scalar_tensor_tensor(s1, bcx[1], xqc[:, 1:2], s0, OP.mult, OP.add)
                    s2 = p_mask.tile([P, LE], f32, tag="s")
                    nc.vector.scalar_tensor_tensor(s2, bcx[2], xqc[:, 2:3], s1, OP.mult, OP.add)
                    s3 = p_mask.tile([P, LE], f32, tag="s")
                    nc.vector.tensor_tensor(s3, s2, bcx[3], OP.subtract)
                    m8 = p_m8.tile([P, 8], f32, tag="m8")
                    nc.vector.max(m8, s3)
                    s4 = p_mask.tile([P, LE], f32, tag="s")
                    nc.vector.match_replace(s4, m8, s3, NEG)
                    m8b = p_m8.tile([P, 8], f32, tag="m8")
                    nc.vector.max(m8b, s4)
                    s5 = p_mask.tile([P, LE], f32, tag="s")
                    nc.vector.match_replace(s5, m8b, s4, NEG)
                    m8c = p_m8.tile([P, 8], f32, tag="m8")
                    nc.vector.max(m8c, s5)
                    # rank-17 value (16 NN + self) is the inclusion threshold;
                    # PE-transpose the per-query column into one PSUM row
                    # (replaces a 512-descriptor DRAM gather that stalled ~50us)
                    nc.tensor.matmul(
                        psT[0:1, qt * P:(qt + 1) * P], m8c[:, 0:1], ident_sb,
                        is_transpose=True, start=qt == 0, stop=qt == 3,
                        skip_group_check=True,
                    )
                t_row = pc.tile([1, TOWN], f32)
                nc.vector.tensor_copy(t_row, psT)
                t2 = pc.tile([1, TOWN], f32)
                nc.vector.tensor_tensor(t2, t_row, bos_sb, OP.min)
                psBC = ps_mm.tile([P, TOWN], f32, tag="mm")
                nc.tensor.matmul(psBC, ones_row, t2, start=True, stop=True)
                t_bc = pc.tile([P, TOWN], f32)
                nc.vector.tensor_copy(t_bc, psBC)
                bq = []
                for c in range(3):
                    t = bcast_rows(xq2row_d[c], p_mbc, TOWN, tag=f"bq{c}")
                    bq.append(t)
                for kt in range(NKT):
                    xkc = p_m8.tile([P, 4], f32, tag="xkc")
                    nc.sync.dma_start(out=xkc, in_=xkn_d[kt * P:(kt + 1) * P, :])
                    u0 = p_mask.tile([P, TOWN], f32, tag="st")
                    nc.vector.tensor_scalar(u0, bq[0], xkc[:, 0:1], None, op0=OP.mult)
                    u1 = p_mask.tile([P, TOWN], f32, tag="st")
                    nc.vector.scalar_tensor_tensor(u1, bq[1], xkc[:, 1:2], u0, OP.mult, OP.add)
                    u2 = p_mask.tile([P, TOWN], f32, tag="st")
                    nc.vector.scalar_tensor_tensor(u2, bq[2], xkc[:, 2:3], u1, OP.mult, OP.add)
                    u3 = p_mask.tile([P, TOWN], f32, tag="st")
                    nc.vector.tensor_scalar(u3, u2, xkc[:, 3:4], None, op0=OP.subtract)
                    al = p_allow.tile([P, TOWN], bf16, tag="allow")
                    nc.vector.tensor_tensor(al, u3, t_bc, OP.is_ge)
                    if kt == 0:
                        # BOS key allowed for all q; emitted here (not after the
                        # loop) so attention on kt=0 can start immediately
                        nc.vector.memset(al[0:1, :], 1.0)
                    if dbga_t is not None:
                        nc.sync.dma_start(out=dbgs_t[kt], in_=u3)
                    allow_sb.append(al)
                if dbga_t is not None:
                    nc.sync.dma_start(out=dbgt_t[:, :], in_=t2)
                    for kt in range(NKT):
                        nc.sync.dma_start(out=dbga_t[kt], in_=allow_sb[kt])
            return allow_sb



        # ================= helpers =================
        def load_w(pool, dram_ap, kchunks, cols, tag):
            t = pool.tile([P, kchunks, cols], bf16, tag=tag)
            nc.sync.dma_start(
                out=t, in_=dram_ap.rearrange("(kc p) m -> p kc m", p=P)
            )
            return t

        def xshadow(x):
            # bf16 shadow + square, emitted at x-production time so the next
            # layer_norm starts directly with its reduction matmuls
            xb = p_lnsq.tile([P, TOWN], bf16, tag="lnxb")
            nc.vector.tensor_copy(xb, x)
            sq = p_lnsq.tile([P, TOWN], bf16, tag="lnsq")
            nc.scalar.activation(sq, x, AF.Square)
            return xb, sq

        def layer_norm(xst, out_dt, out_pool, out_tag):
            xs, xbs, sqs = xst
            ps_mean = ps_mm.tile([1, TOWN], f32, tag="mm")
            for dt in range(NDT):
                nc.tensor.matmul(ps_mean, ones_col_bf, xbs[dt], start=dt == 0, stop=dt == 3)
            ps_sq = ps_mm.tile([1, TOWN], f32, tag="mm")
            for dt in range(NDT):
                nc.tensor.matmul(ps_sq, ones_col_bf, sqs[dt], start=dt == 0, stop=dt == 3)
            mu16 = p_small.tile([1, TOWN], f16, tag="smf16")
            nc.scalar.mul(mu16, ps_mean, 1.0 / D)
            musq = p_small.tile([1, TOWN], f32, tag="sm")
            nc.scalar.activation(musq, ps_mean, AF.Square, scale=1.0 / D)
            var = p_small.tile([1, TOWN], f32, tag="sm")
            nc.vector.scalar_tensor_tensor(var, ps_sq, 1.0 / D, musq, OP.mult, OP.subtract)
            lnv = p_small.tile([1, TOWN], f32, tag="sm")
            nc.scalar.activation(lnv, var, AF.Ln, bias=eps_sb)
            rstd = p_small.tile([1, TOWN], f16, tag="smf16")
            nc.scalar.activation(rstd, lnv, AF.Exp, scale=-0.5)
            # h = (x - mu)*rstd via f16 rank-1 broadcasts of mu and rstd
            ps_a = ps_mm.tile([P, TOWN], f32, tag="mm")
            nc.tensor.matmul(ps_a, ones_row_f16, rstd, start=True, stop=True)
            ps_c = ps_mm.tile([P, TOWN], f32, tag="mm")
            nc.tensor.matmul(ps_c, ones_row_f16, mu16, start=True, stop=True)
            a_sb = p_lnac.tile([P, TOWN], f32, tag="lna")
            nc.vector.tensor_copy(a_sb, ps_a)
            c_sb = p_lnac.tile([P, TOWN], f32, tag="lnc")
            nc.vector.tensor_copy(c_sb, ps_c)
            pe_warm(10)  # fill the PE during the stats tail + h ops
            hs = []
            for dt in range(NDT):
                h = out_pool.tile([P, TOWN], out_dt, tag=out_tag)
                nc.vector.tensor_tensor(h, xs[dt], c_sb, OP.subtract)
                (nc.vector if NOGP else nc.gpsimd).tensor_tensor(h, h, a_sb, OP.mult)
                hs.append(h)
            return hs

        def proj_fm(w_sb, col_off, n_m, rhs, bias_ap, out_pool, out_tag, out_dt=bf16):
            """Feature-major projection; per-partition bias applied on eviction."""
            outs = []
            nk = len(rhs)
            for m in range(n_m):
                ps = ps_mm.tile([P, TOWN], f32, tag="mm")
                for kc in range(nk):
                    nc.tensor.matmul(
                        ps, w_sb[:, kc, col_off + m * P:col_off + (m + 1) * P],
                        rhs[kc], start=kc == 0, stop=kc == nk - 1,
                    )
                bcol = p_bias.tile([P, 1], f32, tag="bcol")
                nc.sync.dma_start(out=bcol, in_=bias_ap[col_off + m * P:col_off + (m + 1) * P, :])
                o = out_pool.tile([P, TOWN], out_dt, tag=out_tag)
                nc.vector.tensor_scalar(o, ps, bcol, None, op0=OP.add)
                outs.append(o)
            return outs

        def proj_rank1bias(w_sb, col_off, rhs, bias_ap, bias_off, token_major):
            """K^T (feature-major) or V (token-major) projection with the bias
            folded in as a rank-1 bf16 matmul; evicted compact bf16 for the AG bounce."""
            outs = []
            brow512 = None
            if token_major:
                brow512 = p_bias.tile([1, TOWN], bf16, tag="brow512")
                nc.sync.dma_start(
                    out=brow512,
                    in_=bias_ap[bias_off:bias_off + D, :].rearrange("a b -> b a"),
                )
            for m in range(4):
                ps = ps_mm.tile([P, TOWN], f32, tag="mm")
                if token_major:
                    for kc in range(4):
                        nc.tensor.matmul(
                            ps, rhs[kc][:, m * P:(m + 1) * P],
                            w_sb[:, kc, col_off:col_off + D],
                            start=kc == 0, stop=False,
                        )
                    nc.tensor.matmul(ps, ones_row_bf, brow512, start=False, stop=True)
                else:
                    for kc in range(4):
                        nc.tensor.matmul(
                            ps, w_sb[:, kc, col_off + m * P:col_off + (m + 1) * P],
                            rhs[kc], start=kc == 0, stop=False,
                        )
                    brow = p_bias.tile([1, P], bf16, tag="brow")
                    nc.sync.dma_start(
                        out=brow,
                        in_=bias_ap[bias_off + m * P:bias_off + (m + 1) * P, :].rearrange("a b -> b a"),
                    )
                    nc.tensor.matmul(ps, brow, ones_row512_bf, start=False, stop=True)
                o = p_kvout.tile([P, TOWN], bf16, tag="kvout")
                nc.vector.tensor_copy(o, ps)
                outs.append(o)
            return outs

        def kv_allgather(k_outs, v_outs, uniq):
            # one DRAM buffer set per layer: a pool-recycled buffer could be
            # rewritten while the pair-peer's collective still reads it
            # (cross-core hazard invisible to Tile's per-core dep tracking).
            # K and V go in separate AllGathers so attention (which needs K
            # first) can start while V is still in flight.
            bin_k = p_dram.tile([4, P, TOWN], bf16, tag=f"agink{uniq}")
            for p in range(4):
                nc.sync.dma_start(out=bin_k[p], in_=k_outs[p])
            bout_k = p_dram.tile([2, 4, P, TOWN], bf16, tag=f"agoutk{uniq}")
            nc.gpsimd.collective_compute(
                "AllGather", OP.bypass, replica_groups=PAIRS,
                ins=[bin_k[:].opt()], outs=[bout_k[:].opt()],
            )
            bin_v = p_dram.tile([4, P, TOWN], bf16, tag=f"aginv{uniq}")
            for p in range(4):
                nc.sync.dma_start(out=bin_v[p], in_=v_outs[p])
            bout_v = p_dram.tile([2, 4, P, TOWN], bf16, tag=f"agoutv{uniq}")
            nc.gpsimd.collective_compute(
                "AllGather", OP.bypass, replica_groups=PAIRS,
                ins=[bin_v[:].opt()], outs=[bout_v[:].opt()],
            )
            return (bout_k, bout_v)

        def load_k(bout):
            Ks = []
            for p in range(4):
                kt = p_kv.tile([P, 2, TOWN], bf16, tag="ksb")
                for r in range(2):
                    nc.sync.dma_start(out=kt[:, r, :], in_=bout[r, p])
                Ks.append(kt)
            return Ks

        def load_v(bout):
            Vs = []
            for r in range(2):
                for tt in range(4):
                    vt = p_v.tile([P, H, 65], bf16, tag="vsb")
                    nc.sync.dma_start(
                        out=vt[:, :, 0:64],
                        in_=bout[r, tt].rearrange("p (h d) -> p h d", h=H),
                    )
                    (nc.vector if NOGP else nc.gpsimd).memset(vt[:, :, 64:65], 1.0)
                    Vs.append(vt)
            return Vs

        def attention(Qs, Ks, Vs, mask_tiles):
            # Scores for the head pair land in one 2-bank PSUM tile [P, 2*TOWN]
            # (j=0 cols 0:TOWN, j=1 cols TOWN:2*TOWN) so exp and the mask
            # multiply run once per (hp, kt) at N=1024 instead of twice at
            # N=512 — the serial exp->mask chain was the attention rate
            # limiter. psO is evicted to SBUF with a single copy so the two
            # PSUM banks recycle quickly.
            OTs = []
            for hp in range(4):
                psO = []
                for _j in range(2):
                    psO_t = ps_o.tile([65, TOWN], f32, tag="pso")
                    psO.append(psO_t)
                for kt in range(NKT):
                    r, c = kt // 4, kt % 4
                    psP = ps_s.tile([P, 2 * TOWN], f32, tag="pss")
                    for j in range(2):
                        rows = slice(j * 64, (j + 1) * 64)
                        nc.tensor.matmul(
                            psP[:, j * TOWN:(j + 1) * TOWN],
                            Ks[hp][rows, r, c * P:(c + 1) * P], Qs[hp][rows, :],
                            start=True, stop=True,
                        )
                    e = p_e.tile([P, 2 * TOWN], bf16, tag="e")
                    nc.scalar.activation(e, psP, AF.Exp, scale=0.125)
                    if mask_tiles is not None:
                        # one multiply over both halves; the [P, TOWN] mask is
                        # repeated via a stride-0 middle AP dim
                        m_ap = mask_tiles[kt][:, :]
                        m_rep = cbass.AP(
                            tensor=m_ap.tensor, offset=m_ap.offset,
                            ap=[list(m_ap.ap[0]), [0, 2]] + [list(d) for d in m_ap.ap[1:]],
                        )
                        e_ap = e[:, :]
                        e_v = cbass.AP(
                            tensor=e_ap.tensor, offset=e_ap.offset,
                            ap=[list(e_ap.ap[0]), [TOWN, 2], [1, TOWN]],
                        )
                        nc.vector.tensor_tensor(e_v, e_v, m_rep, OP.mult)
                    for j in range(2):
                        head = 2 * hp + j
                        nc.tensor.matmul(
                            psO[j], Vs[kt][:, head, :], e[:, j * TOWN:(j + 1) * TOWN],
                            start=kt == 0, stop=kt == NKT - 1,
                        )
                ot = p_ot.tile([P, TOWN], bf16, tag="ot")
                for j in range(2):
                    oraw = p_oraw.tile([65, TOWN], f32, tag="oraw")
                    nc.vector.tensor_copy(oraw, psO[j])
                    den = p_small.tile([1, TOWN], f32, tag="sm")
                    nc.vector.tensor_copy(den, oraw[64:65, :])
                    rec = p_small.tile([1, TOWN], f32, tag="sm")
                    nc.vector.reciprocal_approx_fast(rec, den)
                    if hp < 3:
                        # broadcast rec across 64 partitions via a DRAM
                        # round-trip (frees PE + DVE; the latency hides under
                        # the next head-pair's kt loop)
                        recd = p_dram.tile([1, TOWN], f32, tag="recd")
                        nc.sync.dma_start(out=recd, in_=rec)
                        bc = p_bc.tile([64, TOWN], f32, tag="bc")
                        rap = recd[0]
                        nc.sync.dma_start(out=bc, in_=cbass.AP(
                            tensor=rap.tensor, offset=rap.offset,
                            ap=[[0, 64]] + [list(d) for d in rap.ap],
                        ))
                    else:
                        # last pair: out-proj waits on this chain and the PE is
                        # idle, so the rank-1 matmul is the lower-latency path
                        psB = ps_mm.tile([64, TOWN], f32, tag="mm")
                        nc.tensor.matmul(psB, ones_row[:, 0:64], rec, start=True, stop=True)
                        bc = p_bc.tile([64, TOWN], f32, tag="bc")
                        nc.vector.tensor_copy(bc, psB)
                    nc.vector.tensor_tensor(ot[j * 64:(j + 1) * 64, :], oraw[0:64, :], bc, OP.mult)
                OTs.append(ot)
            return OTs

        def proj_residual(w_sb, col_off, n_k, rhs, bias_ap, xs):
            nxs, nxbs, nsqs = [], [], []
            for m in range(NDT):
                ps = ps_mm.tile([P, TOWN], f32, tag="mm")
                for kc in range(n_k):
                    nc.tensor.matmul(
                        ps, w_sb[:, kc, col_off + m * P:col_off + (m + 1) * P],
                        rhs[kc], start=kc == 0, stop=kc == n_k - 1,
                    )
                bcol = p_bias.tile([P, 1], f32, tag="bcol")
                nc.sync.dma_start(out=bcol, in_=bias_ap[m * P:(m + 1) * P, :])
                nx = p_x.tile([P, TOWN], f32, tag="x")
                nc.vector.scalar_tensor_tensor(nx, ps, bcol, xs[m], OP.add, OP.add)
                nxs.append(nx)
                xb, sq = xshadow(nx)
                nxbs.append(xb); nsqs.append(sq)
            return nxs, nxbs, nsqs

        def ffn(w1_ap, w2_ap, b1_ap, b2_ap, hs, xs):
            nxbs, nsqs = [], []
            gs = []
            for m in range(F // P):
                w1m = p_w1.tile([P, NDT, P], bf16, tag="wf1")
                nc.sync.dma_start(
                    out=w1m,
                    in_=w1_ap[:, m * P:(m + 1) * P].rearrange("(kc p) m -> p kc m", p=P),
                )
                ps = ps_mm.tile([P, TOWN], f32, tag="mm")
                for kc in range(NDT):
                    nc.tensor.matmul(
                        ps, w1m[:, kc, :], hs[kc],
                        start=kc == 0, stop=kc == NDT - 1,
                    )
                bcol = p_bias.tile([P, 1], f32, tag="bcol")
                nc.sync.dma_start(out=bcol, in_=b1_ap[m * P:(m + 1) * P, :])
                g = p_g.tile([P, TOWN], bf16, tag="g")
                nc.scalar.activation(g, ps, AF.Gelu, bias=bcol)
                gs.append(g)
            nxs = []
            for m in range(NDT):
                w2m = p_w2.tile([P, F // P, P], bf16, tag="wf2")
                nc.sync.dma_start(
                    out=w2m,
                    in_=w2_ap[:, m * P:(m + 1) * P].rearrange("(kc p) c -> p kc c", p=P),
                )
                ps2 = ps_mm.tile([P, TOWN], f32, tag="mm")
                for kc in range(F // P):
                    nc.tensor.matmul(
                        ps2, w2m[:, kc, :], gs[kc],
                        start=kc == 0, stop=kc == F // P - 1,
                    )
                bcol = p_bias.tile([P, 1], f32, tag="bcol")
                nc.sync.dma_start(out=bcol, in_=b2_ap[m * P:(m + 1) * P, :])
                nx = p_x.tile([P, TOWN], f32, tag="x")
                nc.vector.scalar_tensor_tensor(nx, ps2, bcol, xs[m], OP.add, OP.add)
                nxs.append(nx)
                xb, sq = xshadow(nx)
                nxbs.append(xb); nsqs.append(sq)
            return nxs, nxbs, nsqs

        p_x = ep(tc.tile_pool(name="p_x", bufs=5))
        p_h = ep(tc.tile_pool(name="p_h", bufs=6))
        p_q = ep(tc.tile_pool(name="p_q", bufs=5))
        p_kv = ep(tc.tile_pool(name="p_kv", bufs=4))
        p_v = ep(tc.tile_pool(name="p_v", bufs=8))
        p_kvout = ep(tc.tile_pool(name="p_kvout", bufs=4))
        p_ot = ep(tc.tile_pool(name="p_ot", bufs=4))
        p_e = ep(tc.tile_pool(name="p_e", bufs=2))
        p_oraw = ep(tc.tile_pool(name="p_oraw", bufs=2))
        p_g = ep(tc.tile_pool(name="p_g", bufs=16))
        p_lnsq = ep(tc.tile_pool(name="p_lnsq", bufs=4))
        p_lnac = ep(tc.tile_pool(name="p_lnac", bufs=1))
        p_bc = ep(tc.tile_pool(name="p_bc", bufs=2))
        p_small = ep(tc.tile_pool(name="p_small", bufs=3))
        p_bias = ep(tc.tile_pool(name="p_bias", bufs=4))
        p_eo = ep(tc.tile_pool(name="p_eo", bufs=4))
        p_eob = ep(tc.tile_pool(name="p_eob", bufs=4))
        p_w1 = ep(tc.tile_pool(name="p_w1", bufs=3))
        p_w2 = ep(tc.tile_pool(name="p_w2", bufs=2))
        p_wqkv = ep(tc.tile_pool(name="p_wqkv", bufs=1))
        p_wout = ep(tc.tile_pool(name="p_wout", bufs=2))

        # ================= encoder =================
        xs = []
        xbs, sqs = [], []
        for dt in range(NDT):
            x = p_x.tile([P, TOWN], f32, tag="x")
            nc.sync.dma_start(out=x, in_=x0T[dt])
            xs.append(x)
            xb, sq = xshadow(x)
            xbs.append(xb); sqs.append(sq)
        xst = (xs, xbs, sqs)

        for l in range(n_enc):
            wqkv = load_w(p_wqkv, ew_qkv[l], NDT, 3 * D, "wqkv")
            wout = load_w(p_wout, ew_out[l], NDT, D, "wout")

            hs = layer_norm(xst, bf16, p_h, "h")
            kouts = proj_rank1bias(wqkv, D, hs, eb_qkv_bf[l], D, token_major=False)
            vouts = proj_rank1bias(wqkv, 2 * D, hs, eb_qkv_bf[l], 2 * D, token_major=True)
            boutk, boutv = kv_allgather(kouts, vouts, f"e{l}")
            Qs = proj_fm(wqkv, 0, 4, hs, eb_qkv[l], p_q, "q")
            if l == 0:
                # decoder layer-0 self-attention K/V depends only on dec_in:
                # project + fire its AllGather here (after e0's AG so the
                # collective queue serves e0 first), then build the kNN mask.
                # All of it overlaps the e0 AllGather wait + mask window.
                yt0 = []
                ybt0, sqt0 = [], []
                for dt in range(NDT):
                    y = p_eo.tile([P, TOWN], f32, tag="eof")
                    nc.sync.dma_start(out=y, in_=y0T[dt])
                    yt0.append(y)
                    yb_, sq_ = xshadow(y)
                    ybt0.append(yb_); sqt0.append(sq_)
                hs_d0 = layer_norm((yt0, ybt0, sqt0), bf16, p_eob, "eob")
                wq_d0 = load_w(p_wqkv, dw_saqkv[0], NDT, 3 * D, "wqkv")
                k_d0 = proj_rank1bias(wq_d0, D, hs_d0, db_saqkv_bf[0], D, token_major=False)
                v_d0 = proj_rank1bias(wq_d0, 2 * D, hs_d0, db_saqkv_bf[0], 2 * D, token_major=True)
                d0_bouts = kv_allgather(k_d0, v_d0, "d0")
                hd0_dram = p_dram.tile([NDT, P, TOWN], bf16, tag="hd0")
                for dt in range(NDT):
                    nc.sync.dma_start(out=hd0_dram[dt], in_=hs_d0[dt])
                allow_sb = build_mask()
            pe_warm(60)
            Ks = load_k(boutk)
            Vs = load_v(boutv)
            if dbgkv_t is not None and l == 1:
                for p in range(4):
                    nc.sync.dma_start(out=dbgk_t[p], in_=Ks[p])
                    nc.sync.dma_start(out=dbgko_t[p], in_=kouts[p])
            OTs = attention(Qs, Ks, Vs, allow_sb)
            xst = proj_residual(wout, 0, NDT, OTs, eb_out[l], xst[0])
            dbg(xst[0])
            hs = layer_norm(xst, bf16, p_h, "h")
            xst = ffn(ew_f1[l], ew_f2[l], eb_f1[l], eb_f2[l], hs, xst[0])
            dbg(xst[0])

        pe_warm(12)
        eof = layer_norm(xst, f32, p_eo, "eof")
        eob = []
        for dt in range(NDT):
            nc.sync.dma_start(out=enc_part[dt], in_=eof[dt])
            t = p_eob.tile([P, TOWN], bf16, tag="eob")
            nc.vector.tensor_copy(t, eof[dt])
            eob.append(t)

        # ==== cross-attention K/V: project now, AllGather per decoder layer ====
        bin_cas = []
        with tc.tile_pool(name="p_wca", bufs=1) as p_wca:
            for l in range(n_dec):
                wkv = p_wca.tile([P, NDT, 2 * D], bf16, tag="wcakv")
                nc.sync.dma_start(
                    out=wkv,
                    in_=dw_caqkv[l][:, D:3 * D].rearrange("(kc p) m -> p kc m", p=P),
                )
                kouts = proj_rank1bias(wkv, 0, eob, db_caqkv_bf[l], D, token_major=False)
                vouts = proj_rank1bias(wkv, D, eob, db_caqkv_bf[l], 2 * D, token_major=True)
                bin_k = p_dram.tile([4, P, TOWN], bf16, tag=f"caink{l}")
                bin_v = p_dram.tile([4, P, TOWN], bf16, tag=f"cainv{l}")
                for p in range(4):
                    nc.sync.dma_start(out=bin_k[p], in_=kouts[p])
                    nc.sync.dma_start(out=bin_v[p], in_=vouts[p])
                bin_cas.append((bin_k, bin_v))

        bout_cas = []
        for l in range(n_dec):
            bk = p_dram.tile([2, 4, P, TOWN], bf16, tag=f"caoutk{l}")
            bv = p_dram.tile([2, 4, P, TOWN], bf16, tag=f"caoutv{l}")
            bout_cas.append((bk, bv))
        fire_cross_ag0_early = True

        def fire_cross_ag(l):
            for i in range(2):
                nc.gpsimd.collective_compute(
                    "AllGather", OP.bypass, replica_groups=PAIRS,
                    ins=[bin_cas[l][i][:].opt()], outs=[bout_cas[l][i][:].opt()],
                )

        # ================= decoder =================
        ys = []
        ybs, qsq = [], []
        for dt in range(NDT):
            y = p_x.tile([P, TOWN], f32, tag="x")
            nc.sync.dma_start(out=y, in_=y0T[dt])
            ys.append(y)
            yb, sq_ = xshadow(y)
            ybs.append(yb); qsq.append(sq_)
        yst = (ys, ybs, qsq)

        fire_cross_ag(0)
        for l in range(n_dec):
            wqkv = load_w(p_wqkv, dw_saqkv[l], NDT, 3 * D, "wqkv")
            wout = load_w(p_wout, dw_saout[l], NDT, D, "wout")

            # self-attention (causal)
            if l == 0:
                # K/V AllGather was prefired at program start; reload h
                hs = []
                for dt in range(NDT):
                    h = p_h.tile([P, TOWN], bf16, tag="h")
                    nc.sync.dma_start(out=h, in_=hd0_dram[dt])
                    hs.append(h)
                boutk, boutv = d0_bouts
            else:
                hs = layer_norm(yst, bf16, p_h, "h")
                kouts = proj_rank1bias(wqkv, D, hs, db_saqkv_bf[l], D, token_major=False)
                vouts = proj_rank1bias(wqkv, 2 * D, hs, db_saqkv_bf[l], 2 * D, token_major=True)
                boutk, boutv = kv_allgather(kouts, vouts, f"d{l}")
            Qs = proj_fm(wqkv, 0, 4, hs, db_saqkv[l], p_q, "q")
            pe_warm(60)
            Ks = load_k(boutk)
            Vs = load_v(boutv)
            OTs = attention(Qs, Ks, Vs, causal_sb)
            if l + 1 < n_dec:
                fire_cross_ag(l + 1)
            bout_ca = bout_cas[l]
            yst = proj_residual(wout, 0, NDT, OTs, db_saout[l], yst[0])
            dbg(yst[0])

            # cross-attention (no mask)
            wcaq = load_w(p_wout, dw_caqkv[l][:, 0:D], NDT, D, "wout")
            wcao = load_w(p_wout, dw_caout[l], NDT, D, "wout")
            hs = layer_norm(yst, bf16, p_h, "h")
            Qs = proj_fm(wcaq, 0, 4, hs, db_caqkv[l], p_q, "q")
            Ks = load_k(bout_ca[0])
            Vs = load_v(bout_ca[1])
            OTs = attention(Qs, Ks, Vs, None)
            yst = proj_residual(wcao, 0, NDT, OTs, db_caout[l], yst[0])
            dbg(yst[0])

            # ffn
            hs = layer_norm(yst, bf16, p_h, "h")
            yst = ffn(dw_f1[l], dw_f2[l], db_f1[l], db_f2[l], hs, yst[0])
            dbg(yst[0])

        dof = layer_norm(yst, f32, p_eo, "eof")
        for dt in range(NDT):
            nc.sync.dma_start(out=dec_part[dt], in_=dof[dt])

    nc.compile()
    return nc


def make_in_maps(inputs):
    inp = {k: np.asarray(v) for k, v in inputs.items()}
    f32 = np.float32

    W = {
        "ew_qkv": np.ascontiguousarray(inp["e_qkv_w"].swapaxes(1, 2)).astype(BF16),
        "ew_out": np.ascontiguousarray(inp["e_out_w"].swapaxes(1, 2)).astype(BF16),
        "ew_f1": np.ascontiguousarray(inp["e_ff1_w"].swapaxes(1, 2)).astype(BF16),
        "ew_f2": np.ascontiguousarray(inp["e_ff2_w"].swapaxes(1, 2)).astype(BF16),
        "eb_qkv": inp["e_qkv_b"].astype(f32).reshape(NE, 3 * D, 1),
        "eb_out": inp["e_out_b"].astype(f32).reshape(NE, D, 1),
        "eb_f1": inp["e_ff1_b"].astype(f32).reshape(NE, F, 1),
        "eb_f2": inp["e_ff2_b"].astype(f32).reshape(NE, D, 1),
        "dw_saqkv": np.ascontiguousarray(inp["d_sa_qkv_w"].swapaxes(1, 2)).astype(BF16),
        "db_saqkv": inp["d_sa_qkv_b"].astype(f32).reshape(ND, 3 * D, 1),
        "dw_saout": np.ascontiguousarray(inp["d_sa_out_w"].swapaxes(1, 2)).astype(BF16),
        "db_saout": inp["d_sa_out_b"].astype(f32).reshape(ND, D, 1),
        "dw_caqkv": np.ascontiguousarray(inp["d_ca_qkv_w"].swapaxes(1, 2)).astype(BF16),
        "db_caqkv": inp["d_ca_qkv_b"].astype(f32).reshape(ND, 3 * D, 1),
        "dw_caout": np.ascontiguousarray(inp["d_ca_out_w"].swapaxes(1, 2)).astype(BF16),
        "db_caout": inp["d_ca_out_b"].astype(f32).reshape(ND, D, 1),
        "dw_f1": np.ascontiguousarray(inp["d_ff1_w"].swapaxes(1, 2)).astype(BF16),
        "db_f1": inp["d_ff1_b"].astype(f32).reshape(ND, F, 1),
        "dw_f2": np.ascontiguousarray(inp["d_ff2_w"].swapaxes(1, 2)).astype(BF16),
        "db_f2": inp["d_ff2_b"].astype(f32).reshape(ND, D, 1),
        "eb_qkv_bf": inp["e_qkv_b"].astype(BF16).reshape(NE, 3 * D, 1),
        "db_saqkv_bf": inp["d_sa_qkv_b"].astype(BF16).reshape(ND, 3 * D, 1),
        "db_caqkv_bf": inp["d_ca_qkv_b"].astype(BF16).reshape(ND, 3 * D, 1),
        "ident": np.eye(P, dtype=np.float32),
    }

    in_maps = []
    for c in range(NCORE):
        b, half = c // 2, c % 2
        sl = slice(half * TOWN, (half + 1) * TOWN)
        m = dict(W)
        xT = np.ascontiguousarray(inp["enc_in"][b].astype(f32).T[:, sl])
        m["x0T"] = xT.reshape(NDT, P, TOWN)
        yT = np.ascontiguousarray(inp["dec_in"][b].astype(f32).T[:, sl])
        m["y0T"] = yT.reshape(NDT, P, TOWN)
        xyz = inp["enc_xyz"][b].astype(f32)
        n2 = (xyz * xyz).sum(-1, dtype=f32).astype(f32)
        xq2 = (np.float32(2.0) * xyz[sl]).astype(f32)
        m["xq2"] = np.ascontiguousarray(xq2)
        m["xq2row"] = np.ascontiguousarray(xq2.T)
        xkn = np.concatenate([xyz, n2[:, None]], 1).astype(f32)
        m["xkn"] = np.ascontiguousarray(xkn)
        m["xrow"] = np.ascontiguousarray(xkn.T)
        bos = np.full((1, TOWN), 1e30, f32)
        if half == 0:
            bos[0, 0] = NEG
        m["bosrow"] = bos
        qg = np.arange(half * TOWN, (half + 1) * TOWN)
        kg = np.arange(LE)
        m["causal"] = np.ascontiguousarray(
            (kg[:, None] <= qg[None, :]).astype(BF16)
        ).reshape(NKT, P, TOWN)
        in_maps.append(m)
    return in_maps


def assemble(results):
    enc = np.zeros((B, LE, D), np.float32)
    dec = np.zeros((B, LD, D), np.float32)
    for c, r in enumerate(results):
        b, half = c // 2, c % 2
        sl = slice(half * TOWN, (half + 1) * TOWN)
        enc[b, sl, :] = r["enc_part"].reshape(D, TOWN).T
        dec[b, sl, :] = r["dec_part"].reshape(D, TOWN).T
    return enc, dec


def kernel(**inputs):
    from concourse import bass_utils

    if "nc" not in _CACHE:
        _CACHE["nc"] = build()
    nc = _CACHE["nc"]
    in_maps = make_in_maps(inputs)
    enc = dec = None
    for attempt in range(3):
        try:
            res = bass_utils.run_bass_kernel_spmd(
                nc, in_maps, core_ids=list(range(NCORE))
            )
        except Exception:
            if attempt == 2:
                raise
            continue
        enc, dec = assemble(res.results)
        # transient first-execution flakes have shown up as NaN output;
        # the math can never produce NaN, so retry on detection
        if not (np.isnan(enc).any() or np.isnan(dec).any()):
            break
    return enc, dec

